# revision 1
# baseline (speedup 1.0000x reference)
"""Multi-head attention (B=2, S=2048, E=1024, H=16) on 8 Trainium2 NeuronCores.

Sharding: core c -> batch c//4, heads 4*(c%4)..4*(c%4)+3  (data + head parallel).
Each core computes a partial output projection [S, E] over its 256 head-dims;
the host sums the 4 bf16 partials per batch in f32 and adds the output bias
(the "all-reduce" happens in the unshard step).

On-chip layouts (contraction always on the partition dim, no on-chip
transposes; host pre-transposes query/key/value):
  QT, KT  [dim, S]   = Wx^T @ X^T      (rhs = X^T chunks streamed from HBM)
  V       [S, dim+ones]                 (natural; a ones column per head makes
                                         the PV matmul also emit softmax sums)
  scoresT [keys, q]  = KT_tile^T-block @ QT     per (head, q-group, key-tile),
                       column-trimmed to the causally live range
  probsT  = exp(scoresT)*lam^8 (bf16)   unnormalized; masked lanes zeroed by a
                                         post-exp 0/1 multiply (Pool) / memset
  attnoutT [d, q]    = (V|1)^T @ probsT (row 64 = Z = sum of probs)
  Zinv    = DVE reciprocal of the Z row -> bf16 -> K=1 broadcast matmul
  partial [S, E]     = attnoutT^T-chunks @ Wo-rows (bf16 out, host-summed)

Matmul operands are bf16 (full PE rate); all accumulation is fp32 in PSUM.
fp8 DoubleRow was evaluated and rejected: e4m3 quantization of X/W/V/attn
each push the absmax-rel error past the 2e-2 gate (host emulation), and this
walrus build cannot codegen CUSTOM_DVE_ANT ops (no DVE-exp offload either).

Schedule (TimelineSim-profiled; PE busy ~117us of ~147us total):
  phase A streams xq -> QT(m0,m1), xk -> KT(m0), then the six earliest
  attention items' scores+exp run on the otherwise-idle ACT while xv streams
  (their PV waits for V in phase B); KT(m1) and the V projection close
  phase A, with V psum copies on DVE (ACT still drains early exps).  Phase B drives a single
  global FIFO of deferred jobs (PV matmuls, Zinv chains, output projections)
  drained 21 jobs behind the eagerly-emitted scores/exp stream, so the
  ACT exp pipeline never stalls at item boundaries.  Work order: the two
  mid-size groups close the kernel with their output projections
  interleaved; diagonal-pair reordering trims exp/matmul columns; psum:
  scores 2x2 banks, pv 2, shared bps/out-proj ring 2.
"""

import sys

for _p in ("/opt/trn_rl_repo", "/root/.axon_site/_ro/trn_rl_repo"):
    if _p not in sys.path:
        sys.path.insert(0, _p)

import numpy as np


# ---------------------------------------------------------------------------
# Patch: the walrus build in this container rejects >1 sem wait on one CTRL
# instruction ("Too many sync wait commands") and the TileContext exit drain
# aggregates every outstanding proc's wait onto a single Drain. Spill the
# excess waits onto SP nops (1 wait each) emitted right after the drain.
# ---------------------------------------------------------------------------
def _install_tile_drain_patch():
    import concourse.tile as tile
    import concourse.mybir as mybir
    from concourse.vector_clock import ScopedClock

    if getattr(tile.TileContext, "_drain_patch_installed", False):
        return

    def _patched_drain_and_barrier(self, tick_clock, wait_clock):
        drain_inst = self.nc.sync.drain()
        wait_clock.add_sem_waits(
            drain_inst.ins, ScopedClock({None: tick_clock.global_clock})
        )
        si = drain_inst.ins.sync_info
        waits = list(si.on_wait) if si and si.on_wait else []
        if len(waits) > 1:
            si.on_wait = waits[:1]
            for w in waits[1:]:
                nop = self.nc.sync.nop(nofuse=True, hint="drain_wait_spill")
                nop.ins.sync_info = mybir.SyncInfo(on_wait=[w], on_update=[])
        self.nc.all_engine_barrier()
        assert self.sems is not None
        popped = self.nc._tile_sem_poison_stack.pop()
        assert popped is self._sem_poison
        self.nc.clear_and_free_semaphores(list(self.sems.allocated().values()))
        self.nc.all_engine_barrier()

    tile.TileContext._drain_and_barrier = _patched_drain_and_barrier
    tile.TileContext._drain_patch_installed = True


def _split_multi_waits(nc, maxw=1):
    """Walrus here allows only `maxw` sem-wait commands per instruction.
    Hoist excess waits onto engine-queue NoOps inserted just before the
    instruction (the sequencer executes them in order, so semantics are
    identical)."""
    import concourse.mybir as mybir

    ctr = 0
    for bb in nc.main_func.blocks:
        new = []
        for inst in bb.instructions:
            si = inst.sync_info
            waits = list(si.on_wait) if si and si.on_wait else []
            if len(waits) > maxw:
                extras = waits[:-maxw]
                si.on_wait = waits[-maxw:]
                for i in range(0, len(extras), maxw):
                    nop = mybir.InstNoOp(
                        name=f"I-waitspill-{ctr}", engine=inst.engine,
                        ins=[], outs=[])
                    ctr += 1
                    nop.sync_info = mybir.SyncInfo(
                        on_wait=extras[i:i + maxw], on_update=[])
                    try:
                        nc.register_instruction(nop, overwrite=True)
                    except Exception:
                        pass
                    new.append(nop)
            new.append(inst)
        bb.instructions = new


# ---------------------------------------------------------------------------
# Custom DVE exp: out = p(s)^8 with p(s) = ((C0 s + C1)^2 + C2) *
# ((C0 s + C3)^2 + 1), a relative-minimax quartic for lam*e^(s/8) on
# [-9, 9] (max |score| is ~8.3).  p^8 = lam^8 * e^s * (1 +- 7e-3); the
# ACT-exp columns get bias 8*ln(lam) so both engines produce identically
# scaled probs (softmax normalization absorbs the lam^8).  The DVE
# pipeline is 8 ALU ops deep, so the op is split: quartic (8 ops) +
# three squarings (3 ops).
# ---------------------------------------------------------------------------
EXPC0 = 0.052498333
EXPC1 = 0.80805624
EXPC2 = 0.11811068
EXPC3 = 0.1425715
EXPLAM = 0.7873084
_EXP_OPS = None


def _get_exp_ops():
    global _EXP_OPS
    if _EXP_OPS is not None:
        return _EXP_OPS
    from concourse.dve_spec import Spec, Src0, Src1, C0, C1, C2, One, sq, lower
    from concourse import dve_ops
    from concourse.dve_uop import DveOpSpec
    from concourse.dve_ops import DveOp

    def ref_q(in0, in1, s0, s1, imm2):
        f = np.float32
        s = np.asarray(in0, np.float32)
        c3 = np.asarray(in1, np.float32).reshape(s.shape[0], -1)[:, 0:1]
        m1 = (f(s0) * s).astype(np.float32)
        A = (((m1 + f(s1)).astype(np.float32)) ** 2).astype(np.float32)
        A = (A + f(imm2)).astype(np.float32)
        Bq = (((m1 + c3).astype(np.float32)) ** 2).astype(np.float32)
        Bq = (Bq + f(1.0)).astype(np.float32)
        return (A * Bq).astype(np.float32)

    def ref_sq8(in0, in1, s0, s1, imm2):
        p = np.asarray(in0, np.float32)
        p = (p * p).astype(np.float32)
        p = (p * p).astype(np.float32)
        return (p * p).astype(np.float32)

    m1 = Src0 * C0
    body_q = (sq(m1 + C1) + C2) * (sq(m1 + Src1) + One)
    body_s = sq(sq(sq(Src0)))
    made = []
    for nm, body, ref, rd1 in (("EXP8Q_ANT", body_q, ref_q, True),
                               ("SQ8_ANT", body_s, ref_sq8, False)):
        if nm in dve_ops._SUB_OPCODE_FOR_NAME:
            made.append(next(o for o in dve_ops.OPS if o.name == nm))
            continue
        opcode = dve_ops._CUSTOM_DVE_ROW_BASE + len(dve_ops.OPS)
        spec = Spec(body=body, reference=ref)
        shas = {}
        for ver in ("v3", "v4"):
            shas[ver] = DveOpSpec(name=nm, opcode=opcode,
                                  uops=lower(spec, ver=ver),
                                  rd1_en=rd1).sha(ver)
        op = DveOp(nm, spec, subdim=False, uops_sha=shas)
        dve_ops.OPS.append(op)
        dve_ops.CUSTOM_DVE_SPECS[nm] = op.spec
        dve_ops._SUB_OPCODE_FOR_NAME[nm] = opcode
        made.append(op)
    _EXP_OPS = tuple(made)
    return _EXP_OPS


# ---------------------------------------------------------------------------
# Mask classification (host side, from the actual mask array).
# Blocks are 128x128 in the *transposed* score layout: block (kt, qb) covers
# keys kt*128.. x queries qb*128... Returns per-block bias indices into a
# stack of unique additive-bias blocks (0 where attended, -1e9 where masked).
# ---------------------------------------------------------------------------
def classify_mask(mask2d, S, KB=128):
    nb = S // KB
    assert mask2d.shape == (S, S)
    assert mask2d.any(axis=1).all(), "a query row with no attended key"
    maskT = mask2d.T  # [keys, q]
    uniq = {}
    biases = []
    bias_idx = {}  # (kt, qb) -> None (all attended) or index
    block_live = np.zeros((nb, nb), dtype=bool)  # any attended key in block
    for kt in range(nb):
        for qb in range(nb):
            blk = maskT[kt * KB:(kt + 1) * KB, qb * KB:(qb + 1) * KB]
            if blk.all():
                bias_idx[(kt, qb)] = None
                block_live[kt, qb] = True
            else:
                b = np.where(blk, np.float32(1.0), np.float32(0.0))
                key = b.tobytes()
                if key not in uniq:
                    uniq[key] = len(biases)
                    biases.append(b)
                bias_idx[(kt, qb)] = uniq[key]
                block_live[kt, qb] = blk.any()
    return bias_idx, biases, block_live


# ---------------------------------------------------------------------------
# Bass program builder (one SPMD program, same for all cores).
# ---------------------------------------------------------------------------
def build_nc(S, E, D, HL, bias_idx, block_live, nuniq, shift=32.0, repeat=1):
    import concourse.bass as bass
    import concourse.mybir as mybir
    import concourse.tile as tile

    f32 = mybir.dt.float32
    bf16 = mybir.dt.bfloat16
    Act = mybir.ActivationFunctionType

    P = 128
    EC = E // P              # E chunks (contraction tiles for projections)
    DIM = HL * D             # this core's head dims (256)
    MT = DIM // P            # m-tiles of QT/KT (2)
    QG = 512                 # q-group width
    NQG = S // QG
    NKT = S // P             # key tiles
    NST = S // P             # s tiles
    VW = HL * (D + 1)        # V width incl. ones columns (260)
    EGW = min(QG, E)         # output E slice width
    NEG = E // EGW           # output E slices (2)

    # key tiles needed per q-group
    def kts_for_group(g):
        out = []
        for kt in range(NKT):
            if any(block_live[kt, g * (QG // P) + j] for j in range(QG // P)):
                out.append(kt)
        return out

    # first live column (within the group's QG window) for a key tile:
    # columns before it belong to fully-masked blocks and are skipped by
    # the scores matmul / exp / PV accumulation.
    def qstart(kt, g):
        for j in range(QG // P):
            if block_live[kt, g * (QG // P) + j]:
                return j * P
        return QG

    nc = bass.Bass()
    dp = nc.declare_dram_parameter
    d_xq = dp("xqT", [E, S], bf16, isOutput=False)
    d_xk = dp("xkT", [E, S], bf16, isOutput=False)
    d_xv = dp("xvT", [E, S], bf16, isOutput=False)
    d_wq = dp("wq", [E, DIM], bf16, isOutput=False)
    d_wk = dp("wk", [E, DIM], bf16, isOutput=False)
    d_wv = dp("wv", [E, VW], bf16, isOutput=False)
    d_wo = dp("wo", [DIM, E], bf16, isOutput=False)
    d_bias = dp("biasT", [P, max(nuniq, 1) * P], bf16, isOutput=False)
    d_out = dp("out_p", [S, E], bf16, isOutput=True)

    import contextlib
    with tile.TileContext(nc) as tc, contextlib.ExitStack() as _stk:
        consts = _stk.enter_context(tc.tile_pool(name="consts", bufs=1))

        # weight tiles: [E, n] rearranged so one DMA loads all chunks
        # (chunk e lives at w_sb[:, e, :]).  DMA emission happens inside
        # emit_once, ordered so the first projection's weights land first.
        w_sb = {}
        for nm, width in (("wq", DIM), ("wk", DIM), ("wv", VW)):
            w_sb[nm] = consts.tile([P, EC, width], bf16, name=f"sb_{nm}",
                                   tag=f"sb_{nm}")
        w_dram = {"wq": d_wq, "wk": d_wk, "wv": d_wv}
        wo_sb = [consts.tile([2 * D, E], bf16, name=f"sb_wo{p}",
                             tag=f"sb_wo{p}") for p in range(HL // 2)]
        bias_sb = consts.tile([P, max(nuniq, 1) * P], bf16, name="sb_bias")
        ones1 = consts.tile([1, D], bf16, name="ones1")
        nc.vector.memset(ones1, 1.0)
        exp8q, sq8 = _get_exp_ops()
        c3_sb = consts.tile([P, 1], f32, name="c3_sb")
        nc.vector.memset(c3_sb, EXPC3)
        import math
        expbias = consts.tile([P, 1], f32, name="expbias")
        nc.vector.memset(expbias, float(8.0 * math.log(EXPLAM)))

        def load_w(nm):
            nc.sync.dma_start(
                out=w_sb[nm],
                in_=w_dram[nm][:, :].rearrange("(e p) n -> p e n", p=P))

        def emit_once():
            # persistent projection outputs
            QT = [consts.tile([P, S], bf16, name=f"QT{m}", tag=f"QT{m}")
                  for m in range(MT)]
            KT = [consts.tile([P, S], bf16, name=f"KT{m}", tag=f"KT{m}")
                  for m in range(MT)]
            V = [consts.tile([P, VW], bf16, name=f"V{s}", tag=f"V{s}")
                 for s in range(NST)]
            # attnT stored as head-PAIR tiles [128, QG]: head 2p ->
            # partitions 0..63, head 2p+1 -> 64..127, so the output
            # projection contracts K=128.
            attnP = [[consts.tile([2 * D, QG], bf16, name=f"attnP{p}g{g}",
                                  tag=f"attnP{p}g{g}") for g in range(NQG)]
                     for p in range(HL // 2)]

            # largest q-groups first; the smallest group's first two heads
            # are pulled into phase A (they only need QT[0]/KT[0]), and the
            # two smallest groups interleave at the end so two dependency
            # chains stay in flight during the drain.
            g_order = sorted(range(NQG),
                             key=lambda g: -len(kts_for_group(g)))
            import os
            _early_n = int(os.environ.get("K2_EARLY", "6"))
            _drain_n = int(os.environ.get("K2_DRAIN", "21"))
            _order = os.environ.get("K2_ORDER", "B")
            work = []
            ga, gb = g_order[-2], g_order[-1]
            if _order == "A":
                work += [(gb, 0), (gb, 1)]
                for g in g_order[:-2]:
                    work += [(g, h) for h in range(HL)]
                work += [(ga, 0), (ga, 1)]
                for h in range(2, HL):
                    work += [(ga, h), (gb, h)]
            elif _order == "B":
                # tail anchored by the second-largest group
                work += [(gb, 0), (gb, 1), (gb, 2), (gb, 3)]
                g0_, g1_, g2_, g3_ = g_order[3], g_order[2], g_order[1], g_order[0]
                work = [(g0_, 0), (g0_, 1)]
                work += [(g3_, h) for h in range(HL)]
                work += [(g0_, 2), (g0_, 3)]
                work += [(g1_, 0), (g1_, 1)]
                work += [(g2_, 0), (g2_, 1), (g2_, 2)]
                work += [(g1_, 2), (g2_, 3), (g1_, 3)]
            else:
                # C: three-way interleaved tail
                work += [(gb, 0), (gb, 1)]
                work += [(g_order[0], h) for h in range(HL)]
                rest = [g_order[1], ga, gb]
                for h in range(HL):
                    for g in rest:
                        if (g, h) not in work:
                            work.append((g, h))
            # early items must be h0/h1: only QT[0]/KT[0] exist at that point
            early = [it for it in work if it[1] < 2][:_early_n]
            # the groups whose output projections close the kernel: their
            # copies go to ACT, which is idle during the drain
            tail_gfin = {g for g, h in work[-3:]}
            last_g = work[-1][0]

            def group_pairs(g):
                kts = kts_for_group(g)
                prs = [kts[i:i + 2] for i in range(0, len(kts), 2)]
                # within a pair, larger qstart first: one exp op then covers
                # [qstart(first) : end] only
                return [sorted(p, key=lambda kt: -qstart(kt, g)) for p in prs]

            with tc.tile_pool(name="probs", bufs=int(__import__("os").environ.get("K2_PROBS", "36"))) as probs_pool, \
                 tc.tile_pool(name="evB", bufs=2) as evB_pool, \
                 tc.tile_pool(name="zrow", bufs=2) as z_pool, \
                 tc.tile_pool(name="outst", bufs=6) as out_pool:

                def emit_pair(g, h, pair, pool, tag, mask_dve=False):
                    """scores + exp + masking for one key-tile pair"""
                    m, po = h // 2, (h % 2) * D
                    w = len(pair) * QG
                    s0 = qstart(pair[0], g)
                    sps = pool.tile([P, 2 * QG], f32, tag=tag, bufs=2,
                                    name="sps")
                    pb = probs_pool.tile([P, 2 * QG], bf16, tag="pb",
                                         name="pb")
                    for x, kt in enumerate(pair):
                        qs = qstart(kt, g)
                        nc.tensor.matmul(
                            sps[:, x * QG + qs:(x + 1) * QG],
                            lhsT=KT[m][po:po + D, kt * P:(kt + 1) * P],
                            rhs=QT[m][po:po + D, g * QG + qs:(g + 1) * QG],
                            start=True, stop=True)
                    nc.scalar.activation(pb[:, s0:w], sps[:, s0:w],
                                         Act.Exp, bias=expbias[:, 0:1])
                    # masking applied AFTER exp (multiplicative 0/1, exact):
                    # partial blocks get a mask multiply on the Pool engine,
                    # fully-masked blocks a gpsimd memset-zero.
                    for x, kt in enumerate(pair):
                        for j in range(QG // P):
                            qb = g * (QG // P) + j
                            bidx = bias_idx[(kt, qb)]
                            if bidx is None:
                                continue
                            blkslice = pb[:, x * QG + j * P:
                                          x * QG + (j + 1) * P]
                            if not block_live[kt, qb]:
                                nc.gpsimd.memset(blkslice, 0.0)
                            elif mask_dve:
                                nc.vector.tensor_mul(
                                    blkslice, blkslice,
                                    bias_sb[:, bidx * P:(bidx + 1) * P])
                            else:
                                nc.gpsimd.tensor_mul(
                                    blkslice, blkslice,
                                    bias_sb[:, bidx * P:(bidx + 1) * P])
                    return pb

                # ---------------- phase A: projections ----------------
                with tc.tile_pool(name="xt", bufs=2 * EC) as xt_pool, \
                     tc.tile_pool(name="psA", bufs=1, space="PSUM") as psA:

                    def stream_chunks(dram):
                        chunks = []
                        for e in range(EC):
                            ch = xt_pool.tile([P, S], bf16, tag="xt",
                                              name=f"xch{e}")
                            nc.sync.dma_start(
                                out=ch, in_=dram[e * P:(e + 1) * P, :])
                            chunks.append(ch)
                        return chunks

                    def proj_qk(wname, dst, chunks, m):
                        pss = [psA.tile([P, QG], f32, tag=f"psA{g}",
                                        name=f"psqk{g}") for g in range(NQG)]
                        for e in range(EC):
                            for g in range(NQG):
                                nc.tensor.matmul(
                                    pss[g],
                                    lhsT=w_sb[wname][:, e, m * P:(m + 1) * P],
                                    rhs=chunks[e][:, g * QG:(g + 1) * QG],
                                    start=(e == 0), stop=(e == EC - 1))
                        for g in range(NQG):
                            nc.vector.tensor_copy(
                                dst[m][:, g * QG:(g + 1) * QG], pss[g])

                    # xq -> QT (both m-tiles, chunk-paced), xk -> KT m0,
                    # then the early items' scores+exp run while xv streams;
                    # KT m1 and the V projection close phase A.
                    load_w("wq")
                    qchunks = stream_chunks(d_xq)
                    load_w("wk")
                    kchunks = stream_chunks(d_xk)
                    proj_qk("wq", QT, qchunks, 0)
                    proj_qk("wq", QT, qchunks, 1)
                    proj_qk("wk", KT, kchunks, 0)

                    load_w("wv")
                    vchunks = stream_chunks(d_xv)
                    for p in range(HL // 2):
                        nc.sync.dma_start(
                            out=wo_sb[p],
                            in_=d_wo[p * 2 * D:(p + 1) * 2 * D, :])
                    nc.sync.dma_start(out=bias_sb, in_=d_bias[:, :])

                    # early attention: scores/exp only (PV waits for V and
                    # runs at the head of phase B).  Early items are h0/h1,
                    # so only QT[0]/KT[0] are needed.
                    early_pend = {}
                    for g, h in early:
                        early_pend[(g, h)] = [
                            (pair, emit_pair(g, h, pair, psA, "se"))
                            for pair in group_pairs(g)]

                    proj_qk("wk", KT, kchunks, 1)

                    # V projection: st-pair outer, e-inner accumulation
                    for stb in range(0, NST, 2):
                        pss = [psA.tile([P, VW], f32, tag=f"psA{i}",
                                        name=f"psv{i}") for i in range(2)]
                        for e in range(EC):
                            for i in range(2):
                                nc.tensor.matmul(
                                    pss[i],
                                    lhsT=vchunks[e][:, (stb + i) * P:
                                                    (stb + i + 1) * P],
                                    rhs=w_sb["wv"][:, e, :],
                                    start=(e == 0), stop=(e == EC - 1))
                        for i in range(2):
                            st = stb + i
                            nc.vector.tensor_copy(V[st], pss[i])
                            onescols = V[st].rearrange(
                                "p (h c) -> p h c", c=D + 1)[:, :, D]
                            nc.vector.memset(onescols, 1.0)

                # ---------------- phase B: attention ----------------
                with tc.tile_pool(name="psS", bufs=2, space="PSUM") as psS, \
                     tc.tile_pool(name="psPV", bufs=2, space="PSUM") as psPV, \
                     tc.tile_pool(name="psX", bufs=2, space="PSUM") as psX:

                    heads_done = {g: 0 for g in range(NQG)}
                    # global FIFO of deferred work (PV matmuls, Zinv chains,
                    # output projections).  Scores/exp are emitted eagerly;
                    # everything downstream drains 3 jobs behind, ACROSS
                    # item boundaries, so the ACT exp stream never waits for
                    # an item's drain.
                    jobs = []

                    def drain(limit):
                        while len(jobs) > limit:
                            jobs.pop(0)()

                    def make_pv_job(pair, pb, g, h, pv, state, nkts):
                        def run():
                            # ascending kt: the first matmul (qstart 0)
                            # initializes the full psum width
                            for x, kt in sorted(enumerate(pair),
                                                key=lambda t: t[1]):
                                qs = qstart(kt, g)
                                nc.tensor.matmul(
                                    pv[:, qs:QG],
                                    lhsT=V[kt][:, h * (D + 1):
                                               (h + 1) * (D + 1)],
                                    rhs=pb[:, x * QG + qs:(x + 1) * QG],
                                    start=(state["npv"] == 0),
                                    stop=(state["npv"] == nkts - 1),
                                    skip_group_check=True)
                                state["npv"] += 1
                        return run

                    def make_hfin_job(g, h, pv):
                        po = (h % 2) * D

                        def run():
                            # Zinv: DVE reciprocal of the ones-column row ->
                            # bf16 broadcast matmul -> (ev sbuf) * (bps psum)
                            # multiply into the bf16 attnP tile.
                            ev = evB_pool.tile([D, QG], f32,
                                               tag=f"ev{h % 2}", name="ev")
                            nc.vector.tensor_copy(ev, pv[0:D, :])
                            zb = z_pool.tile([1, QG], bf16, tag="zb",
                                             name="zb")
                            with nc.allow_low_precision(reason="bf16 zinv"):
                                nc.vector.reciprocal(zb, pv[D:D + 1, :])
                            bps = psX.tile([D, QG], f32, tag="x", name="bps")
                            nc.tensor.matmul(
                                bps, lhsT=ones1, rhs=zb,
                                start=True, stop=True)
                            nc.vector.tensor_mul(
                                attnP[h // 2][g][po:po + D, :], ev, bps)
                        return run

                    def make_gfin_job(g):
                        def run():
                            # ---- output projection for this q-group ----
                            for j in range(QG // P):
                                st = g * (QG // P) + j
                                off = j * P
                                ot = out_pool.tile([P, NEG * EGW], bf16,
                                                   tag="ot", name="ot")
                                for eg in range(NEG):
                                    ops = psX.tile([P, EGW], f32, tag="x",
                                                   name="opso")
                                    for p in range(HL // 2):
                                        nc.tensor.matmul(
                                            ops,
                                            lhsT=attnP[p][g][:, off:off + P],
                                            rhs=wo_sb[p][:, eg * EGW:
                                                         (eg + 1) * EGW],
                                            start=(p == 0),
                                            stop=(p == HL // 2 - 1))
                                    # the last two groups alternate their
                                    # output copies across ACT and DVE so
                                    # the drain chain is halved; the rest
                                    # stay on DVE
                                    if g in tail_gfin and eg == 0:
                                        nc.scalar.copy(
                                            ot[:, eg * EGW:(eg + 1) * EGW],
                                            ops)
                                    else:
                                        nc.vector.tensor_copy(
                                            ot[:, eg * EGW:(eg + 1) * EGW],
                                            ops)
                                nc.sync.dma_start(
                                    out=d_out[st * P:(st + 1) * P, :], in_=ot)
                        return run

                    for g, h in work:
                        kts = kts_for_group(g)
                        pv = psPV.tile([D + 1, QG], f32, tag="pv", name="pv")
                        state = {"npv": 0}

                        pend = early_pend.pop((g, h), None)
                        if pend is None:
                            for pair in group_pairs(g):
                                pb = emit_pair(g, h, pair, psS, "s",
                                               mask_dve=g in (ga, gb))
                                jobs.append(make_pv_job(pair, pb, g, h, pv,
                                                        state, len(kts)))
                                drain(_drain_n)
                        else:
                            for pair, pb in pend:
                                jobs.append(make_pv_job(pair, pb, g, h, pv,
                                                        state, len(kts)))
                                drain(_drain_n)

                        jobs.append(make_hfin_job(g, h, pv))
                        heads_done[g] += 1
                        if heads_done[g] == HL:
                            jobs.append(make_gfin_job(g))
                        drain(_drain_n + 1)

                    drain(0)

        for _rep in range(repeat):
            emit_once()

    _split_multi_waits(nc)
    return nc


# ---------------------------------------------------------------------------
# Host entry point
# ---------------------------------------------------------------------------
LAST_EXEC_NS = None
LAST_RESULT = None


def kernel(query, key, value, mask, Wq, Wk, Wv, Wo, bo):
    global LAST_EXEC_NS, LAST_RESULT
    _install_tile_drain_patch()
    from concourse.bass_utils import run_bass_kernel_spmd

    B, S, E = 2, 2048, 1024
    H, D = 16, 64
    N_CORES = 8
    BG = 2                    # batch groups
    HG = N_CORES // BG        # head groups per batch
    HL = H // HG              # heads per core
    DIM = HL * D

    query = np.asarray(query, dtype=np.float32)
    key = np.asarray(key, dtype=np.float32)
    value = np.asarray(value, dtype=np.float32)
    mask2d = np.asarray(mask).reshape(S, S).astype(bool)
    Wq = np.asarray(Wq, dtype=np.float32)
    Wk = np.asarray(Wk, dtype=np.float32)
    Wv = np.asarray(Wv, dtype=np.float32)
    Wo = np.asarray(Wo, dtype=np.float32)
    bo = np.asarray(bo, dtype=np.float32)

    bias_idx, biases, block_live = classify_mask(mask2d, S)
    nuniq = len(biases)
    bias_stack = (np.concatenate(biases, axis=1) if nuniq
                  else np.zeros((128, 128), np.float32))

    nc = build_nc(S, E, D, HL, bias_idx, block_live, nuniq)

    scale = np.float32(1.0 / np.sqrt(D))
    in_maps = []
    for c in range(N_CORES):
        b, hg = c // HG, c % HG
        cols = slice(hg * DIM, (hg + 1) * DIM)
        wv_l = Wv[:, cols].reshape(E, HL, D)
        wv_aug = np.zeros((E, HL, D + 1), np.float32)
        wv_aug[:, :, :D] = wv_l
        in_maps.append({
            "xqT": _bf16(query[b].T),
            "xkT": _bf16(key[b].T),
            "xvT": _bf16(value[b].T),
            "wq": _bf16(Wq[:, cols] * scale),
            "wk": _bf16(Wk[:, cols]),
            "wv": _bf16(wv_aug.reshape(E, HL * (D + 1))),
            "wo": _bf16(Wo[cols, :]),
            "biasT": _bf16(bias_stack),
        })

    res = run_bass_kernel_spmd(nc, in_maps, list(range(N_CORES)))
    LAST_RESULT = res
    LAST_EXEC_NS = res.exec_time_ns or res.mean_exec_time_ns

    out = np.empty((B, S, E), np.float32)
    for b in range(BG):
        acc = res.results[b * HG]["out_p"].astype(np.float32)
        for j in range(1, HG):
            acc = acc + res.results[b * HG + j]["out_p"]
        out[b] = acc + bo[None, :]
    return out


def _bf16(a):
    import ml_dtypes
    return np.ascontiguousarray(np.asarray(a, np.float32)).astype(
        ml_dtypes.bfloat16)



# revision 44
# speedup vs baseline: 1.1349x; 1.1349x over previous
"""Multi-head attention (B=2, S=2048, E=1024, H=16) on 8 Trainium2 NeuronCores.

Sharding: core c -> batch c//4, heads 4*(c%4)..4*(c%4)+3  (data + head parallel).
Each core computes a partial output projection [S, E] over its 256 head-dims;
the host sums the 4 bf16 partials per batch in f32 and adds the output bias
(the "all-reduce" happens in the unshard step).

On-chip layouts (contraction always on the partition dim; host pre-transposes
query/key/value):
  QT, KT  [dim, S]   = Wx^T @ X^T      (rhs = X^T chunks streamed from HBM)
  V       [S, dim+ones]                 (natural; a ones column per head makes
                                         the PV matmul also emit softmax sums)
  scoresT [keys, q]  = KT_tile^T-block @ QT     per (head, q-group, key-tile),
                       column-trimmed to the causally live range
  probsT  = exp(scoresT) (bf16)         unnormalized; diagonal blocks masked by
                                        a post-exp 0/1 multiply (Pool/DVE);
                                        fully-dead blocks are simply never read
  pvq     [q, 4*(d+1)] = probs-block^T-stationary @ (V|1)   per 128-q block,
                       accumulated over key tiles; col d=64 is Z = sum of probs.
                       (Probs-stationary puts the SMALL dim (65) on the moving
                       free axis: 35.4k PE columns vs 69.6k the other way.)
  zinv    [q, 1]     = f32 DVE reciprocal of the Z column (per-partition scalar)
  attnN   [q, 2d]    = pvq * zinv (DVE tensor_scalar, head pair packed)
  attnP   [2d, q]    = PE transpose of attnN (identity matmul) + psum->sbuf copy
  partial [S, E]     = attnP^T-chunks @ Wo-rows (bf16 out, host-summed)

Matmul operands are bf16 (full PE rate); all accumulation is fp32 in PSUM.
fp8 DoubleRow was evaluated and rejected: e4m3 quantization of X/W/V/attn
each push the absmax-rel error past the 2e-2 gate (host emulation).

Cost-model notes (TimelineSim): matmul cost = out-free-columns x pe_cycle,
independent of K and M, Ldweights free.  Hence PV runs probs-stationary
(128 q on the PE partitions, 65 moving columns) and Zinv is a per-partition
scalar instead of a broadcast matmul.  PE columns: scores 69.6k + PV 35.4k +
proj 98.8k + outproj 32.8k + transposes 4.1k ~= 240.7k -> ~100us PE busy.
DMA is single-slot: ~52us serialized; first wq chunk and first xq half-chunk
are split out so the first projection matmul starts ~1.5us in.
"""

import sys

for _p in ("/opt/trn_rl_repo", "/root/.axon_site/_ro/trn_rl_repo"):
    if _p not in sys.path:
        sys.path.insert(0, _p)

import numpy as np


# ---------------------------------------------------------------------------
# Patch: the walrus build in this container rejects >1 sem wait on one CTRL
# instruction ("Too many sync wait commands") and the TileContext exit drain
# aggregates every outstanding proc's wait onto a single Drain. Spill the
# excess waits onto SP nops (1 wait each) emitted right after the drain.
# ---------------------------------------------------------------------------
def _install_tile_drain_patch():
    import concourse.tile as tile
    import concourse.mybir as mybir
    from concourse.vector_clock import ScopedClock

    if getattr(tile.TileContext, "_drain_patch_installed", False):
        return

    def _patched_drain_and_barrier(self, tick_clock, wait_clock):
        drain_inst = self.nc.sync.drain()
        wait_clock.add_sem_waits(
            drain_inst.ins, ScopedClock({None: tick_clock.global_clock})
        )
        si = drain_inst.ins.sync_info
        waits = list(si.on_wait) if si and si.on_wait else []
        if len(waits) > 1:
            si.on_wait = waits[:1]
            for w in waits[1:]:
                nop = self.nc.sync.nop(nofuse=True, hint="drain_wait_spill")
                nop.ins.sync_info = mybir.SyncInfo(on_wait=[w], on_update=[])
        self.nc.all_engine_barrier()
        assert self.sems is not None
        popped = self.nc._tile_sem_poison_stack.pop()
        assert popped is self._sem_poison
        self.nc.clear_and_free_semaphores(list(self.sems.allocated().values()))
        self.nc.all_engine_barrier()

    tile.TileContext._drain_and_barrier = _patched_drain_and_barrier
    tile.TileContext._drain_patch_installed = True


def _split_multi_waits(nc, maxw=1):
    """Walrus here allows only `maxw` sem-wait commands per instruction.
    Hoist excess waits onto engine-queue NoOps inserted just before the
    instruction (the sequencer executes them in order, so semantics are
    identical)."""
    import concourse.mybir as mybir

    ctr = 0
    for bb in nc.main_func.blocks:
        new = []
        for inst in bb.instructions:
            si = inst.sync_info
            waits = list(si.on_wait) if si and si.on_wait else []
            if len(waits) > maxw:
                extras = waits[:-maxw]
                si.on_wait = waits[-maxw:]
                for i in range(0, len(extras), maxw):
                    nop = mybir.InstNoOp(
                        name=f"I-waitspill-{ctr}", engine=inst.engine,
                        ins=[], outs=[])
                    ctr += 1
                    nop.sync_info = mybir.SyncInfo(
                        on_wait=extras[i:i + maxw], on_update=[])
                    try:
                        nc.register_instruction(nop, overwrite=True)
                    except Exception:
                        pass
                    new.append(nop)
            new.append(inst)
        bb.instructions = new


# ---------------------------------------------------------------------------
# Mask classification (host side, from the actual mask array).
# Blocks are 128x128 in the *transposed* score layout: block (kt, qb) covers
# keys kt*128.. x queries qb*128... Returns per-block bias indices into a
# stack of unique multiplicative 0/1 mask blocks.
# ---------------------------------------------------------------------------
def classify_mask(mask2d, S, KB=128):
    nb = S // KB
    assert mask2d.shape == (S, S)
    assert mask2d.any(axis=1).all(), "a query row with no attended key"
    maskT = mask2d.T  # [keys, q]
    uniq = {}
    biases = []
    bias_idx = {}  # (kt, qb) -> None (all attended) or index
    block_live = np.zeros((nb, nb), dtype=bool)  # any attended key in block
    for kt in range(nb):
        for qb in range(nb):
            blk = maskT[kt * KB:(kt + 1) * KB, qb * KB:(qb + 1) * KB]
            if blk.all():
                bias_idx[(kt, qb)] = None
                block_live[kt, qb] = True
            else:
                b = np.where(blk, np.float32(1.0), np.float32(0.0))
                key = b.tobytes()
                if key not in uniq:
                    uniq[key] = len(biases)
                    biases.append(b)
                bias_idx[(kt, qb)] = uniq[key]
                block_live[kt, qb] = blk.any()
    return bias_idx, biases, block_live


# ---------------------------------------------------------------------------
# Bass program builder (one SPMD program, same for all cores).
# ---------------------------------------------------------------------------
def build_nc(S, E, D, HL, bias_idx, block_live, nuniq, repeat=1):
    import concourse.bass as bass
    import concourse.mybir as mybir
    import concourse.tile as tile
    from concourse.masks import make_identity

    f32 = mybir.dt.float32
    bf16 = mybir.dt.bfloat16
    Act = mybir.ActivationFunctionType

    P = 128
    EC = E // P              # E chunks (contraction tiles for projections)
    DIM = HL * D             # this core's head dims (256)
    MT = DIM // P            # m-tiles of QT/KT (2)
    QG = 512                 # q-group width
    JB = QG // P             # 128-q blocks per group (4)
    NQG = S // QG
    NKT = S // P             # key tiles
    NST = S // P             # s tiles
    D1 = D + 1
    VW = HL * D1             # V width incl. ones columns (260)
    EGW = min(QG, E)         # output E slice width
    NEG = E // EGW           # output E slices (2)

    # key tiles needed per q-group
    def kts_for_group(g):
        out = []
        for kt in range(NKT):
            if any(block_live[kt, g * JB + j] for j in range(JB)):
                out.append(kt)
        return out

    # first live column (within the group's QG window) for a key tile
    def qstart(kt, g):
        for j in range(JB):
            if block_live[kt, g * JB + j]:
                return j * P
        return QG

    def live_kts(g, j):
        return [kt for kt in kts_for_group(g) if block_live[kt, g * JB + j]]

    nc = bass.Bass()
    dp = nc.declare_dram_parameter
    d_xq = dp("xqT", [E, S], bf16, isOutput=False)
    d_xk = dp("xkT", [E, S], bf16, isOutput=False)
    d_xv = dp("xvT", [E, S], bf16, isOutput=False)
    d_wq = dp("wq", [E, DIM], bf16, isOutput=False)
    d_wk = dp("wk", [E, DIM], bf16, isOutput=False)
    d_wv = dp("wv", [E, VW], bf16, isOutput=False)
    d_wo = dp("wo", [DIM, E], bf16, isOutput=False)
    d_bias = dp("biasT", [P, max(nuniq, 1) * P], bf16, isOutput=False)
    d_out = dp("out_p", [S, E], bf16, isOutput=True)

    import os
    import contextlib
    with tile.TileContext(nc) as tc, contextlib.ExitStack() as _stk:
        consts = _stk.enter_context(tc.tile_pool(name="consts", bufs=1))

        # weight tiles: [E, n] rearranged so chunk e lives at w_sb[:, e, :].
        w_sb = {}
        for nm, width in (("wq", DIM), ("wk", DIM), ("wv", VW)):
            w_sb[nm] = consts.tile([P, EC, width], bf16, name=f"sb_{nm}",
                                   tag=f"sb_{nm}")
        w_dram = {"wq": d_wq, "wk": d_wk, "wv": d_wv}
        wo_sb = [consts.tile([2 * D, E], bf16, name=f"sb_wo{p}",
                             tag=f"sb_wo{p}") for p in range(HL // 2)]
        bias_sb = consts.tile([P, max(nuniq, 1) * P], bf16, name="sb_bias")
        ident = consts.tile([P, P], bf16, name="ident")
        make_identity(nc, ident)

        def load_w(nm, lo=0, hi=None):
            hi = EC if hi is None else hi
            src = w_dram[nm][:, :].rearrange("(e p) n -> p e n", p=P)
            nc.sync.dma_start(out=w_sb[nm][:, lo:hi, :], in_=src[:, lo:hi, :])

        def emit_once():
            # persistent projection outputs
            QT = [consts.tile([P, S], bf16, name=f"QT{m}", tag=f"QT{m}")
                  for m in range(MT)]
            KT = [consts.tile([P, S], bf16, name=f"KT{m}", tag=f"KT{m}")
                  for m in range(MT)]
            V = [consts.tile([P, VW], bf16, name=f"V{s}", tag=f"V{s}")
                 for s in range(NST)]
            # attnT stored as head-PAIR tiles [128, QG]: head 2p ->
            # partitions 0..63, head 2p+1 -> 64..127, so the output
            # projection contracts K=128.
            attnP = [[consts.tile([2 * D, QG], bf16, name=f"attnP{p}g{g}",
                                  tag=f"attnP{p}g{g}") for g in range(NQG)]
                     for p in range(HL // 2)]

            g_order = sorted(range(NQG),
                             key=lambda g: -len(kts_for_group(g)))
            _drain_n = int(os.environ.get("K2_DRAIN", "64"))
            # g0_ = smallest group ... g3_ = largest
            g0_, g1_, g2_, g3_ = (g_order[3], g_order[2], g_order[1],
                                  g_order[0])
            # Items pre-emitted (scores+exp) during phase A, in three waves:
            #   eA: right after the first column-half of Q/K projections
            #       (these only touch QT/KT columns < S/2)
            #   eB: between the second-half Q and K projections (ditto)
            #   eC: after full projections, filling the PE while xv streams
            eA = [(g0_, 0), (g0_, 1), (g0_, 2)]
            eB = [(g0_, 3), (g1_, 0), (g1_, 1), (g1_, 2), (g1_, 3)]
            eC = [(g3_, 0), (g3_, 1)]
            early = eA + eB + eC
            # B-phase processing order: early items' PV first (consumes the
            # held probs, frees the ring), then the B-scored items; the
            # smallest group's last head anchors a short tail.
            # B-scored items (g3 h2/h3, g2 all) spread one-per-two slots so
            # the exp stream never dries; pre-emitted PV items fill between.
            work = [(g0_, 0), (g0_, 1), (g0_, 2), (g1_, 0), (g1_, 1),
                    (g1_, 2), (g3_, 0), (g3_, 1), (g1_, 3),
                    (g3_, 2), (g2_, 0), (g3_, 3), (g2_, 1),
                    (g2_, 2), (g0_, 3), (g2_, 3)]
            assert sorted(work) == sorted(
                (g, h) for g in range(NQG) for h in range(HL))
            tail_gfin = {g for g, h in work[-3:]}
            last_g = work[-1][0]
            ga, gb = g1_, g0_   # groups whose mask-muls go to DVE

            def group_pairs(g):
                kts = kts_for_group(g)
                prs = [kts[i:i + 2] for i in range(0, len(kts), 2)]
                # within a pair, larger qstart first: one exp op then covers
                # [qstart(first) : end] only
                return [sorted(p, key=lambda kt: -qstart(kt, g)) for p in prs]

            with tc.tile_pool(name="probs", bufs=int(os.environ.get("K2_PROBS", "40"))) as probs_pool, \
                 tc.tile_pool(name="attnN", bufs=int(os.environ.get("K2_ATTN", "12"))) as attnN_pool, \
                 tc.tile_pool(name="zrow", bufs=8) as z_pool, \
                 tc.tile_pool(name="outst", bufs=4) as out_pool:

                def emit_pair(g, h, pair, pool, tag, mask_dve=False):
                    """scores + exp + diagonal masking for one key-tile pair"""
                    m, po = h // 2, (h % 2) * D
                    w = len(pair) * QG
                    s0 = qstart(pair[0], g)
                    sps = pool.tile([P, 2 * QG], f32, tag=tag, bufs=2,
                                    name="sps")
                    pb = probs_pool.tile([P, 2 * QG], bf16, tag="pb",
                                         name="pb")
                    for x, kt in enumerate(pair):
                        qs = qstart(kt, g)
                        nc.tensor.matmul(
                            sps[:, x * QG + qs:(x + 1) * QG],
                            lhsT=KT[m][po:po + D, kt * P:(kt + 1) * P],
                            rhs=QT[m][po:po + D, g * QG + qs:(g + 1) * QG],
                            start=True, stop=True)
                    qs1 = qstart(pair[1], g) if len(pair) > 1 else 0
                    if len(pair) > 1 and qs1 > 0:
                        # skip the dead hole [QG : QG+qs1] (never written)
                        nc.scalar.activation(pb[:, s0:QG], sps[:, s0:QG],
                                             Act.Exp)
                        nc.scalar.activation(pb[:, QG + qs1:w],
                                             sps[:, QG + qs1:w], Act.Exp)
                    else:
                        nc.scalar.activation(pb[:, s0:w], sps[:, s0:w],
                                             Act.Exp)
                    # masking applied AFTER exp (multiplicative 0/1, exact)
                    # on partially-live (diagonal) blocks only; fully-dead
                    # blocks are never read by the PV matmuls.
                    for x, kt in enumerate(pair):
                        for j in range(JB):
                            qb = g * JB + j
                            bidx = bias_idx[(kt, qb)]
                            if bidx is None or not block_live[kt, qb]:
                                continue
                            blkslice = pb[:, x * QG + j * P:
                                          x * QG + (j + 1) * P]
                            if mask_dve:
                                nc.vector.tensor_mul(
                                    blkslice, blkslice,
                                    bias_sb[:, bidx * P:(bidx + 1) * P])
                            else:
                                nc.gpsimd.tensor_mul(
                                    blkslice, blkslice,
                                    bias_sb[:, bidx * P:(bidx + 1) * P])
                    return pb

                # ---------------- phase A: projections ----------------
                # Column-phased: Q/K are projected for the first S/2 query
                # columns (both m-tiles) as soon as those half-chunks land,
                # so scores+exp for the low-column groups start ~15us in.
                HS = S // 2
                early_pend = {}

                def emit_early(items, pool):
                    for g, h in items:
                        early_pend[(g, h)] = [
                            (pair, emit_pair(g, h, pair, pool, "se"))
                            for pair in group_pairs(g)]

                with tc.tile_pool(name="xt", bufs=2 * EC) as xt_pool, \
                     tc.tile_pool(name="psA", bufs=1, space="PSUM") as psA:

                    def mk_chunks(nm):
                        return [xt_pool.tile([P, S], bf16, tag="xt",
                                             name=f"{nm}{e}")
                                for e in range(EC)]

                    qch, kch = mk_chunks("q"), mk_chunks("k")

                    def dma_half(dram, chunks, lo, hi, skip_e0=False):
                        for e in range(1 if skip_e0 else 0, EC):
                            nc.sync.dma_start(
                                out=chunks[e][:, lo:hi],
                                in_=dram[e * P:(e + 1) * P, lo:hi])

                    # DMA issue order = transfer order (single DMA engine):
                    # minimal prefix first so the first matmul starts ~2.5us.
                    load_w("wq", 0, 1)
                    nc.sync.dma_start(out=qch[0][:, 0:HS],
                                      in_=d_xq[0:P, 0:HS])
                    nc.sync.dma_start(out=qch[1][:, 0:HS],
                                      in_=d_xq[P:2 * P, 0:HS])
                    load_w("wq", 1, EC)
                    for e in range(2, EC):
                        nc.sync.dma_start(out=qch[e][:, 0:HS],
                                          in_=d_xq[e * P:(e + 1) * P, 0:HS])
                    load_w("wk")
                    # the early waves' diagonal mask-muls read bias_sb, so
                    # its DMA must be issued before they are emitted
                    nc.sync.dma_start(out=bias_sb, in_=d_bias[:, :])
                    dma_half(d_xk, kch, 0, HS)
                    dma_half(d_xq, qch, HS, S)
                    dma_half(d_xk, kch, HS, S)
                    load_w("wv")

                    def proj_half(wname, dst, chunks, c0, c1,
                                  interleave=()):
                        gs = list(range(c0 // QG, c1 // QG))
                        itq = list(interleave)
                        pss = {}
                        for m in range(MT):
                            for g in gs:
                                pss[(m, g)] = psA.tile(
                                    [P, QG], f32, tag=f"pj{m}{g % 2}",
                                    name="pspj")
                        for e in range(EC):
                            for m in range(MT):
                                for g in gs:
                                    nc.tensor.matmul(
                                        pss[(m, g)],
                                        lhsT=w_sb[wname][:, e,
                                                         m * P:(m + 1) * P],
                                        rhs=chunks[e][:, g * QG:(g + 1) * QG],
                                        start=(e == 0), stop=(e == EC - 1))
                            # pre-emitted scores between e-steps keep the
                            # exp stream fed while this projection runs
                            if e % 2 == 1 and itq:
                                emit_early([itq.pop(0)], psA)
                        # score-relevant (low-g) copies first so the next
                        # early wave's scores unblock as soon as possible
                        for g in gs:
                            for m in range(MT):
                                nc.vector.tensor_copy(
                                    dst[m][:, g * QG:(g + 1) * QG],
                                    pss[(m, g)])
                        emit_early(itq, psA)

                    proj_half("wq", QT, qch, 0, HS)
                    proj_half("wk", KT, kch, 0, HS)
                    emit_early(eA, psA)
                    proj_half("wq", QT, qch, HS, S, interleave=eB)
                    proj_half("wk", KT, kch, HS, S)

                    # xv reuses the q-chunk slots; issue its DMAs only now
                    # that every qch reader is emitted (WAR ordering).
                    vch = mk_chunks("v")
                    for e in range(EC):
                        nc.sync.dma_start(out=vch[e],
                                          in_=d_xv[e * P:(e + 1) * P, :])
                    for p in range(HL // 2):
                        nc.sync.dma_start(
                            out=wo_sb[p],
                            in_=d_wo[p * 2 * D:(p + 1) * 2 * D, :])

                    emit_early(eC, psA)

                    # V projection: 4 passes of 4 s-tiles, e-OUTER within a
                    # pass so the first pass streams JIT with the arriving
                    # xv chunks instead of waiting for the whole tensor.
                    for vp in range(NST // 4):
                        sts = range(4 * vp, 4 * vp + 4)
                        pss = {st: psA.tile([P, VW], f32,
                                            tag=f"pj{(st % 4) // 2}{st % 2}",
                                            name=f"psv{st}") for st in sts}
                        for e in range(EC):
                            for st in sts:
                                nc.tensor.matmul(
                                    pss[st],
                                    lhsT=vch[e][:, st * P:(st + 1) * P],
                                    rhs=w_sb["wv"][:, e, :],
                                    start=(e == 0), stop=(e == EC - 1))
                        for st in sts:
                            nc.vector.tensor_copy(V[st], pss[st])
                            onescols = V[st].rearrange(
                                "p (h c) -> p h c", c=D1)[:, :, D]
                            nc.gpsimd.memset(onescols, 1.0)

                # ---------------- phase B: attention ----------------
                with tc.tile_pool(name="psS", bufs=2, space="PSUM") as psS, \
                     tc.tile_pool(name="psPV", bufs=2, space="PSUM") as psPV, \
                     tc.tile_pool(name="psX", bufs=2, space="PSUM") as psX:

                    heads_done = {g: 0 for g in range(NQG)}
                    attnN = {}   # (g, p, j) -> sbuf tile [P, 2D]
                    # global FIFO of deferred work (PV matmuls, finishes,
                    # transposes, output projections), drained N jobs behind
                    # the eagerly-emitted scores/exp stream.
                    jobs = []
                    # live probs-ring tiles: pre-emitted pbs count at B start;
                    # each pv job emission frees one slot (emission order is
                    # what matters for deadlock-freedom)
                    alive = [sum(len(v) for v in early_pend.values())]
                    _pb_cap = int(os.environ.get("K2_PROBS", "40")) - 2

                    def drain(limit):
                        while len(jobs) > limit:
                            jobs.pop(0)()

                    def make_pv_job(pair, pb, g, h, pvq, state, total):
                        def run():
                            alive[0] -= 1
                            # ascending kt within the pair.  PSUM start=True
                            # zeroes the whole 2KB zero-region (bank), so
                            # only the very FIRST matmul into this tile may
                            # carry start=True; later first-touches of other
                            # j-regions overwrite via the pending-zero bits.
                            for x, kt in sorted(enumerate(pair),
                                                key=lambda t: t[1]):
                                for j in range(JB):
                                    if not block_live[kt, g * JB + j]:
                                        continue
                                    state["n"] += 1
                                    nc.tensor.matmul(
                                        pvq[:, j * D1:(j + 1) * D1],
                                        lhsT=pb[:, x * QG + j * P:
                                                x * QG + (j + 1) * P],
                                        rhs=V[kt][:, h * D1:(h + 1) * D1],
                                        start=(state["n"] == 1),
                                        stop=(state["n"] == total),
                                        skip_group_check=True)
                        return run

                    def make_hfin_job(g, h, pvq):
                        p, po = h // 2, (h % 2) * D

                        def run():
                            # per q-block: f32 reciprocal of the Z column,
                            # then a per-partition scaled copy into the
                            # head-pair attnN tile (bf16).  For the closing
                            # groups the odd head's copy goes to ACT so both
                            # halves land in parallel.
                            for j in range(JB):
                                if h % 2 == 0:
                                    attnN[(g, p, j)] = attnN_pool.tile(
                                        [P, 2 * D], bf16, tag="an",
                                        name="an")
                                an = attnN[(g, p, j)]
                                zt = z_pool.tile([P, 1], f32, tag="zt",
                                                 name="zt")
                                nc.vector.reciprocal(
                                    zt, pvq[:, j * D1 + D:(j + 1) * D1])
                                nc.vector.tensor_scalar_mul(
                                    an[:, po:po + D],
                                    pvq[:, j * D1:j * D1 + D], zt)
                        return run

                    def make_tr_job(g, p):
                        def run():
                            # transpose the head-pair q-blocks to [2d, q]
                            # (identity matmuls) landing side-by-side in ONE
                            # psum tile, then a single [P, QG] copy to sbuf
                            pst = psX.tile([P, QG], bf16, tag="x",
                                           name="pst")
                            for j in range(JB):
                                nc.tensor.transpose(
                                    pst[:, j * P:(j + 1) * P],
                                    attnN.pop((g, p, j)), ident)
                            nc.vector.tensor_copy(attnP[p][g], pst)
                        return run

                    def make_gfin_job(g, j):
                        def run():
                            # ---- output projection for q-block j of g ----
                            st = g * JB + j
                            off = j * P
                            ot = out_pool.tile([P, NEG * EGW], bf16,
                                               tag="ot", name="ot")
                            for eg in range(NEG):
                                ops = psX.tile([P, EGW], f32, tag="x",
                                               name="opso")
                                for p in range(HL // 2):
                                    nc.tensor.matmul(
                                        ops,
                                        lhsT=attnP[p][g][:, off:off + P],
                                        rhs=wo_sb[p][:, eg * EGW:
                                                     (eg + 1) * EGW],
                                        start=(p == 0),
                                        stop=(p == HL // 2 - 1))
                                # alternate the copies across ACT and DVE
                                # for the closing groups so the drain chain
                                # is halved; the rest stay on DVE
                                if g in tail_gfin and eg == 0:
                                    nc.scalar.copy(
                                        ot[:, eg * EGW:(eg + 1) * EGW],
                                        ops)
                                else:
                                    nc.vector.tensor_copy(
                                        ot[:, eg * EGW:(eg + 1) * EGW],
                                        ops)
                                if g in tail_gfin:
                                    # per-eg DMA so the close-out chain is
                                    # copy->small-DMA instead of full-row
                                    nc.sync.dma_start(
                                        out=d_out[st * P:(st + 1) * P,
                                                  eg * EGW:(eg + 1) * EGW],
                                        in_=ot[:, eg * EGW:(eg + 1) * EGW])
                            if g not in tail_gfin:
                                nc.sync.dma_start(
                                    out=d_out[st * P:(st + 1) * P, :],
                                    in_=ot)
                        return run

                    for g, h in work:
                        pvq = psPV.tile([P, JB * D1], f32, tag="pv",
                                        name="pvq")

                        state = {"n": 0}
                        total = sum(len(live_kts(g, j)) for j in range(JB))
                        pend = early_pend.pop((g, h), None)
                        if pend is None:
                            for pair in group_pairs(g):
                                while alive[0] >= _pb_cap and jobs:
                                    jobs.pop(0)()
                                alive[0] += 1
                                pb = emit_pair(g, h, pair, psS, "s",
                                               mask_dve=g in (ga, gb))
                                jobs.append(make_pv_job(pair, pb, g, h, pvq,
                                                        state, total))
                                drain(_drain_n)
                        else:
                            for pair, pb in pend:
                                jobs.append(make_pv_job(pair, pb, g, h, pvq,
                                                        state, total))
                                drain(int(os.environ.get("K2_DRAIN_E", "48")))

                        jobs.append(make_hfin_job(g, h, pvq))
                        if h % 2 == 1:
                            jobs.append(make_tr_job(g, h // 2))
                        heads_done[g] += 1
                        if heads_done[g] == HL:
                            for j in range(JB):
                                jobs.append(make_gfin_job(g, j))
                        drain(_drain_n + 1)

                    drain(0)

        for _rep in range(repeat):
            emit_once()

    _split_multi_waits(nc)
    return nc


# ---------------------------------------------------------------------------
# Host entry point
# ---------------------------------------------------------------------------
LAST_EXEC_NS = None
LAST_RESULT = None


def kernel(query, key, value, mask, Wq, Wk, Wv, Wo, bo):
    global LAST_EXEC_NS, LAST_RESULT
    _install_tile_drain_patch()
    from concourse.bass_utils import run_bass_kernel_spmd

    B, S, E = 2, 2048, 1024
    H, D = 16, 64
    N_CORES = 8
    BG = 2                    # batch groups
    HG = N_CORES // BG        # head groups per batch
    HL = H // HG              # heads per core
    DIM = HL * D

    query = np.asarray(query, dtype=np.float32)
    key = np.asarray(key, dtype=np.float32)
    value = np.asarray(value, dtype=np.float32)
    mask2d = np.asarray(mask).reshape(S, S).astype(bool)
    Wq = np.asarray(Wq, dtype=np.float32)
    Wk = np.asarray(Wk, dtype=np.float32)
    Wv = np.asarray(Wv, dtype=np.float32)
    Wo = np.asarray(Wo, dtype=np.float32)
    bo = np.asarray(bo, dtype=np.float32)

    bias_idx, biases, block_live = classify_mask(mask2d, S)
    nuniq = len(biases)
    bias_stack = (np.concatenate(biases, axis=1) if nuniq
                  else np.zeros((128, 128), np.float32))

    nc = build_nc(S, E, D, HL, bias_idx, block_live, nuniq)

    scale = np.float32(1.0 / np.sqrt(D))
    in_maps = []
    for c in range(N_CORES):
        b, hg = c // HG, c % HG
        cols = slice(hg * DIM, (hg + 1) * DIM)
        wv_l = Wv[:, cols].reshape(E, HL, D)
        wv_aug = np.zeros((E, HL, D + 1), np.float32)
        wv_aug[:, :, :D] = wv_l
        in_maps.append({
            "xqT": _bf16(query[b].T),
            "xkT": _bf16(key[b].T),
            "xvT": _bf16(value[b].T),
            "wq": _bf16(Wq[:, cols] * scale),
            "wk": _bf16(Wk[:, cols]),
            "wv": _bf16(wv_aug.reshape(E, HL * (D + 1))),
            "wo": _bf16(Wo[cols, :]),
            "biasT": _bf16(bias_stack),
        })

    res = run_bass_kernel_spmd(nc, in_maps, list(range(N_CORES)))
    LAST_RESULT = res
    LAST_EXEC_NS = res.exec_time_ns or res.mean_exec_time_ns

    out = np.empty((B, S, E), np.float32)
    for b in range(BG):
        acc = res.results[b * HG]["out_p"].astype(np.float32)
        for j in range(1, HG):
            acc = acc + res.results[b * HG + j]["out_p"]
        out[b] = acc + bo[None, :]
    return out


def _bf16(a):
    import ml_dtypes
    return np.ascontiguousarray(np.asarray(a, np.float32)).astype(
        ml_dtypes.bfloat16)


# revision 47
# speedup vs baseline: 1.1690x; 1.0301x over previous
"""Multi-head attention (B=2, S=2048, E=1024, H=16) on 8 Trainium2 NeuronCores.

Sharding: core c -> batch c//4, heads 4*(c%4)..4*(c%4)+3  (data + head parallel).
Each core computes a partial output projection [S, E] over its 256 head-dims;
the host sums the 4 bf16 partials per batch in f32 and adds the output bias
(the "all-reduce" happens in the unshard step).

On-chip layouts (contraction always on the partition dim; host pre-transposes
query/key/value):
  QT, KT  [dim, S]   = Wx^T @ X^T      (rhs = X^T chunks streamed from HBM)
  V       [S, dim+ones]                 (natural; a ones column per head makes
                                         the PV matmul also emit softmax sums)
  scoresT [keys, q]  = KT_tile^T-block @ QT     per (head, q-group, key-tile),
                       column-trimmed to the causally live range
  probsT  = exp(scoresT) (bf16)         unnormalized; diagonal blocks masked by
                                        a post-exp 0/1 multiply (Pool/DVE);
                                        fully-dead blocks are simply never read
  pvq     [q, 4*(d+1)] = probs-block^T-stationary @ (V|1)   per 128-q block,
                       accumulated over key tiles; col d=64 is Z = sum of probs.
                       (Probs-stationary puts the SMALL dim (65) on the moving
                       free axis: 35.4k PE columns vs 69.6k the other way.)
  zinv    [q, 1]     = f32 DVE reciprocal of the Z column (per-partition scalar)
  attnN   [q, 2d]    = pvq * zinv (DVE tensor_scalar, head pair packed)
  attnP   [2d, q]    = PE transpose of attnN (identity matmul) + psum->sbuf copy
  partial [S, E]     = attnP^T-chunks @ Wo-rows (bf16 out, host-summed)

Matmul operands are bf16 (full PE rate); all accumulation is fp32 in PSUM.
fp8 DoubleRow was evaluated and rejected: e4m3 quantization of X/W/V/attn
each push the absmax-rel error past the 2e-2 gate (host emulation).

Cost-model notes (TimelineSim): matmul cost = out-free-columns x pe_cycle,
independent of K and M, Ldweights free.  Hence PV runs probs-stationary
(128 q on the PE partitions, 65 moving columns) and Zinv is a per-partition
scalar instead of a broadcast matmul.  PE columns: scores 69.6k + PV 35.4k +
proj 98.8k + outproj 32.8k + transposes 4.1k ~= 240.7k -> ~100us PE busy.
DMA is single-slot: ~52us serialized; first wq chunk and first xq half-chunk
are split out so the first projection matmul starts ~1.5us in.
"""

import sys

for _p in ("/opt/trn_rl_repo", "/root/.axon_site/_ro/trn_rl_repo"):
    if _p not in sys.path:
        sys.path.insert(0, _p)

import numpy as np


# ---------------------------------------------------------------------------
# Patch: the walrus build in this container rejects >1 sem wait on one CTRL
# instruction ("Too many sync wait commands") and the TileContext exit drain
# aggregates every outstanding proc's wait onto a single Drain. Spill the
# excess waits onto SP nops (1 wait each) emitted right after the drain.
# ---------------------------------------------------------------------------
def _install_tile_drain_patch():
    import concourse.tile as tile
    import concourse.mybir as mybir
    from concourse.vector_clock import ScopedClock

    if getattr(tile.TileContext, "_drain_patch_installed", False):
        return

    def _patched_drain_and_barrier(self, tick_clock, wait_clock):
        drain_inst = self.nc.sync.drain()
        wait_clock.add_sem_waits(
            drain_inst.ins, ScopedClock({None: tick_clock.global_clock})
        )
        si = drain_inst.ins.sync_info
        waits = list(si.on_wait) if si and si.on_wait else []
        if len(waits) > 1:
            si.on_wait = waits[:1]
            for w in waits[1:]:
                nop = self.nc.sync.nop(nofuse=True, hint="drain_wait_spill")
                nop.ins.sync_info = mybir.SyncInfo(on_wait=[w], on_update=[])
        self.nc.all_engine_barrier()
        assert self.sems is not None
        popped = self.nc._tile_sem_poison_stack.pop()
        assert popped is self._sem_poison
        self.nc.clear_and_free_semaphores(list(self.sems.allocated().values()))
        self.nc.all_engine_barrier()

    tile.TileContext._drain_and_barrier = _patched_drain_and_barrier
    tile.TileContext._drain_patch_installed = True


def _split_multi_waits(nc, maxw=1):
    """Walrus here allows only `maxw` sem-wait commands per instruction.
    Hoist excess waits onto engine-queue NoOps inserted just before the
    instruction (the sequencer executes them in order, so semantics are
    identical)."""
    import concourse.mybir as mybir

    ctr = 0
    for bb in nc.main_func.blocks:
        new = []
        for inst in bb.instructions:
            si = inst.sync_info
            waits = list(si.on_wait) if si and si.on_wait else []
            if len(waits) > maxw:
                extras = waits[:-maxw]
                si.on_wait = waits[-maxw:]
                for i in range(0, len(extras), maxw):
                    nop = mybir.InstNoOp(
                        name=f"I-waitspill-{ctr}", engine=inst.engine,
                        ins=[], outs=[])
                    ctr += 1
                    nop.sync_info = mybir.SyncInfo(
                        on_wait=extras[i:i + maxw], on_update=[])
                    try:
                        nc.register_instruction(nop, overwrite=True)
                    except Exception:
                        pass
                    new.append(nop)
            new.append(inst)
        bb.instructions = new


# ---------------------------------------------------------------------------
# Mask classification (host side, from the actual mask array).
# Blocks are 128x128 in the *transposed* score layout: block (kt, qb) covers
# keys kt*128.. x queries qb*128... Returns per-block bias indices into a
# stack of unique multiplicative 0/1 mask blocks.
# ---------------------------------------------------------------------------
def classify_mask(mask2d, S, KB=128):
    nb = S // KB
    assert mask2d.shape == (S, S)
    assert mask2d.any(axis=1).all(), "a query row with no attended key"
    maskT = mask2d.T  # [keys, q]
    uniq = {}
    biases = []
    bias_idx = {}  # (kt, qb) -> None (all attended) or index
    block_live = np.zeros((nb, nb), dtype=bool)  # any attended key in block
    for kt in range(nb):
        for qb in range(nb):
            blk = maskT[kt * KB:(kt + 1) * KB, qb * KB:(qb + 1) * KB]
            if blk.all():
                bias_idx[(kt, qb)] = None
                block_live[kt, qb] = True
            else:
                b = np.where(blk, np.float32(1.0), np.float32(0.0))
                key = b.tobytes()
                if key not in uniq:
                    uniq[key] = len(biases)
                    biases.append(b)
                bias_idx[(kt, qb)] = uniq[key]
                block_live[kt, qb] = blk.any()
    return bias_idx, biases, block_live


# ---------------------------------------------------------------------------
# Bass program builder (one SPMD program, same for all cores).
# ---------------------------------------------------------------------------
def build_nc(S, E, D, HL, bias_idx, block_live, nuniq, repeat=1):
    import concourse.bass as bass
    import concourse.mybir as mybir
    import concourse.tile as tile
    from concourse.masks import make_identity

    f32 = mybir.dt.float32
    bf16 = mybir.dt.bfloat16
    Act = mybir.ActivationFunctionType

    P = 128
    EC = E // P              # E chunks (contraction tiles for projections)
    DIM = HL * D             # this core's head dims (256)
    MT = DIM // P            # m-tiles of QT/KT (2)
    QG = 512                 # q-group width
    JB = QG // P             # 128-q blocks per group (4)
    NQG = S // QG
    NKT = S // P             # key tiles
    NST = S // P             # s tiles
    D1 = D + 1
    VW = HL * D1             # V width incl. ones columns (260)
    EGW = min(QG, E)         # output E slice width
    NEG = E // EGW           # output E slices (2)

    # key tiles needed per q-group
    def kts_for_group(g):
        out = []
        for kt in range(NKT):
            if any(block_live[kt, g * JB + j] for j in range(JB)):
                out.append(kt)
        return out

    # first live column (within the group's QG window) for a key tile
    def qstart(kt, g):
        for j in range(JB):
            if block_live[kt, g * JB + j]:
                return j * P
        return QG

    def live_kts(g, j):
        return [kt for kt in kts_for_group(g) if block_live[kt, g * JB + j]]

    nc = bass.Bass()
    dp = nc.declare_dram_parameter
    d_xq = dp("xqT", [E, S], bf16, isOutput=False)
    d_xk = dp("xkT", [E, S], bf16, isOutput=False)
    d_xv = dp("xvT", [E, S], bf16, isOutput=False)
    d_wq = dp("wq", [E, DIM], bf16, isOutput=False)
    d_wk = dp("wk", [E, DIM], bf16, isOutput=False)
    d_wv = dp("wv", [E, VW], bf16, isOutput=False)
    d_wo = dp("wo", [DIM, E], bf16, isOutput=False)
    d_bias = dp("biasT", [P, max(nuniq, 1) * P], bf16, isOutput=False)
    d_out = dp("out_p", [S, E], bf16, isOutput=True)

    import os
    import contextlib
    with tile.TileContext(nc) as tc, contextlib.ExitStack() as _stk:
        consts = _stk.enter_context(tc.tile_pool(name="consts", bufs=1))

        # weight tiles: [E, n] rearranged so chunk e lives at w_sb[:, e, :].
        w_sb = {}
        for nm, width in (("wq", DIM), ("wk", DIM), ("wv", VW)):
            w_sb[nm] = consts.tile([P, EC, width], bf16, name=f"sb_{nm}",
                                   tag=f"sb_{nm}")
        w_dram = {"wq": d_wq, "wk": d_wk, "wv": d_wv}
        wo_sb = [consts.tile([2 * D, E], bf16, name=f"sb_wo{p}",
                             tag=f"sb_wo{p}") for p in range(HL // 2)]
        bias_sb = consts.tile([P, max(nuniq, 1) * P], bf16, name="sb_bias")
        ident = consts.tile([P, P], bf16, name="ident")
        make_identity(nc, ident)

        def load_w(nm, lo=0, hi=None):
            hi = EC if hi is None else hi
            src = w_dram[nm][:, :].rearrange("(e p) n -> p e n", p=P)
            nc.sync.dma_start(out=w_sb[nm][:, lo:hi, :], in_=src[:, lo:hi, :])

        def emit_once():
            # persistent projection outputs
            QT = [consts.tile([P, S], bf16, name=f"QT{m}", tag=f"QT{m}")
                  for m in range(MT)]
            KT = [consts.tile([P, S], bf16, name=f"KT{m}", tag=f"KT{m}")
                  for m in range(MT)]
            V = [consts.tile([P, VW], bf16, name=f"V{s}", tag=f"V{s}")
                 for s in range(NST)]
            # attnT stored as head-PAIR tiles [128, QG]: head 2p ->
            # partitions 0..63, head 2p+1 -> 64..127, so the output
            # projection contracts K=128.
            attnP = [[consts.tile([2 * D, QG], bf16, name=f"attnP{p}g{g}",
                                  tag=f"attnP{p}g{g}") for g in range(NQG)]
                     for p in range(HL // 2)]

            g_order = sorted(range(NQG),
                             key=lambda g: -len(kts_for_group(g)))
            _drain_n = int(os.environ.get("K2_DRAIN", "64"))
            # g0_ = smallest group ... g3_ = largest
            g0_, g1_, g2_, g3_ = (g_order[3], g_order[2], g_order[1],
                                  g_order[0])
            # Items pre-emitted (scores+exp) during phase A, in three waves:
            #   eA: right after the first column-half of Q/K projections
            #       (these only touch QT/KT columns < S/2)
            #   eB: between the second-half Q and K projections (ditto)
            #   eC: after full projections, filling the PE while xv streams
            eA = [(g0_, 0), (g0_, 1), (g0_, 2)]
            eB = [(g0_, 3), (g1_, 0), (g1_, 1), (g1_, 2), (g1_, 3)]
            eC = [(g3_, 0), (g3_, 1)]
            early = eA + eB + eC
            # B-phase processing order: early items' PV first (consumes the
            # held probs, frees the ring), then the B-scored items; the
            # smallest group's last head anchors a short tail.
            # B-scored items (g3 h2/h3, g2 all) spread one-per-two slots so
            # the exp stream never dries; pre-emitted PV items fill between.
            work = [(g0_, 0), (g0_, 1), (g0_, 2), (g1_, 0), (g1_, 1),
                    (g1_, 2), (g3_, 0), (g3_, 1), (g1_, 3),
                    (g3_, 2), (g2_, 0), (g3_, 3), (g2_, 1),
                    (g0_, 3), (g2_, 2), (g2_, 3)]
            assert sorted(work) == sorted(
                (g, h) for g in range(NQG) for h in range(HL))
            tail_gfin = {g for g, h in work[-3:]}
            last_g = work[-1][0]
            ga, gb = g1_, g0_   # groups whose mask-muls go to DVE

            def group_pairs(g):
                kts = kts_for_group(g)
                prs = [kts[i:i + 2] for i in range(0, len(kts), 2)]
                # within a pair, larger qstart first: one exp op then covers
                # [qstart(first) : end] only
                return [sorted(p, key=lambda kt: -qstart(kt, g)) for p in prs]

            with tc.tile_pool(name="probs", bufs=int(os.environ.get("K2_PROBS", "40"))) as probs_pool, \
                 tc.tile_pool(name="attnN", bufs=int(os.environ.get("K2_ATTN", "12"))) as attnN_pool, \
                 tc.tile_pool(name="zrow", bufs=8) as z_pool, \
                 tc.tile_pool(name="outst", bufs=4) as out_pool:

                def emit_pair(g, h, pair, pool, tag, mask_dve=False):
                    """scores + exp + diagonal masking for one key-tile pair"""
                    m, po = h // 2, (h % 2) * D
                    w = len(pair) * QG
                    s0 = qstart(pair[0], g)
                    sps = pool.tile([P, 2 * QG], f32, tag=tag, bufs=2,
                                    name="sps")
                    pb = probs_pool.tile([P, 2 * QG], bf16, tag="pb",
                                         name="pb")
                    for x, kt in enumerate(pair):
                        qs = qstart(kt, g)
                        nc.tensor.matmul(
                            sps[:, x * QG + qs:(x + 1) * QG],
                            lhsT=KT[m][po:po + D, kt * P:(kt + 1) * P],
                            rhs=QT[m][po:po + D, g * QG + qs:(g + 1) * QG],
                            start=True, stop=True)
                    qs1 = qstart(pair[1], g) if len(pair) > 1 else 0
                    if len(pair) > 1 and qs1 > 0:
                        # skip the dead hole [QG : QG+qs1] (never written)
                        nc.scalar.activation(pb[:, s0:QG], sps[:, s0:QG],
                                             Act.Exp)
                        nc.scalar.activation(pb[:, QG + qs1:w],
                                             sps[:, QG + qs1:w], Act.Exp)
                    else:
                        nc.scalar.activation(pb[:, s0:w], sps[:, s0:w],
                                             Act.Exp)
                    # masking applied AFTER exp (multiplicative 0/1, exact)
                    # on partially-live (diagonal) blocks only; fully-dead
                    # blocks are never read by the PV matmuls.
                    for x, kt in enumerate(pair):
                        for j in range(JB):
                            qb = g * JB + j
                            bidx = bias_idx[(kt, qb)]
                            if bidx is None or not block_live[kt, qb]:
                                continue
                            blkslice = pb[:, x * QG + j * P:
                                          x * QG + (j + 1) * P]
                            if mask_dve:
                                nc.vector.tensor_mul(
                                    blkslice, blkslice,
                                    bias_sb[:, bidx * P:(bidx + 1) * P])
                            else:
                                nc.gpsimd.tensor_mul(
                                    blkslice, blkslice,
                                    bias_sb[:, bidx * P:(bidx + 1) * P])
                    return pb

                # ---------------- phase A: projections ----------------
                # Column-phased: Q/K are projected for the first S/2 query
                # columns (both m-tiles) as soon as those half-chunks land,
                # so scores+exp for the low-column groups start ~15us in.
                HS = S // 2
                early_pend = {}

                def emit_early(items, pool):
                    for g, h in items:
                        early_pend[(g, h)] = [
                            (pair, emit_pair(g, h, pair, pool, "se"))
                            for pair in group_pairs(g)]

                with tc.tile_pool(name="xt", bufs=2 * EC) as xt_pool, \
                     tc.tile_pool(name="psA", bufs=1, space="PSUM") as psA:

                    def mk_chunks(nm):
                        return [xt_pool.tile([P, S], bf16, tag="xt",
                                             name=f"{nm}{e}")
                                for e in range(EC)]

                    qch, kch = mk_chunks("q"), mk_chunks("k")

                    def dma_half(dram, chunks, lo, hi, skip_e0=False):
                        for e in range(1 if skip_e0 else 0, EC):
                            nc.sync.dma_start(
                                out=chunks[e][:, lo:hi],
                                in_=dram[e * P:(e + 1) * P, lo:hi])

                    # DMA issue order = transfer order (single DMA engine):
                    # minimal prefix first so the first matmul starts ~2.5us.
                    load_w("wq", 0, 1)
                    nc.sync.dma_start(out=qch[0][:, 0:HS],
                                      in_=d_xq[0:P, 0:HS])
                    nc.sync.dma_start(out=qch[1][:, 0:HS],
                                      in_=d_xq[P:2 * P, 0:HS])
                    load_w("wq", 1, EC)
                    for e in range(2, EC):
                        nc.sync.dma_start(out=qch[e][:, 0:HS],
                                          in_=d_xq[e * P:(e + 1) * P, 0:HS])
                    load_w("wk")
                    # the early waves' diagonal mask-muls read bias_sb, so
                    # its DMA must be issued before they are emitted
                    nc.sync.dma_start(out=bias_sb, in_=d_bias[:, :])
                    dma_half(d_xk, kch, 0, HS)
                    dma_half(d_xq, qch, HS, S)
                    dma_half(d_xk, kch, HS, S)
                    load_w("wv")

                    def proj_half(wname, dst, chunks, c0, c1,
                                  interleave=()):
                        gs = list(range(c0 // QG, c1 // QG))
                        itq = list(interleave)
                        pss = {}
                        for m in range(MT):
                            for g in gs:
                                pss[(m, g)] = psA.tile(
                                    [P, QG], f32, tag=f"pj{m}{g % 2}",
                                    name="pspj")
                        for e in range(EC):
                            for m in range(MT):
                                for g in gs:
                                    nc.tensor.matmul(
                                        pss[(m, g)],
                                        lhsT=w_sb[wname][:, e,
                                                         m * P:(m + 1) * P],
                                        rhs=chunks[e][:, g * QG:(g + 1) * QG],
                                        start=(e == 0), stop=(e == EC - 1))
                            # pre-emitted scores between e-steps keep the
                            # exp stream fed while this projection runs
                            if e % 2 == 1 and itq:
                                emit_early([itq.pop(0)], psA)
                        # score-relevant (low-g) copies first so the next
                        # early wave's scores unblock as soon as possible
                        for g in gs:
                            for m in range(MT):
                                nc.vector.tensor_copy(
                                    dst[m][:, g * QG:(g + 1) * QG],
                                    pss[(m, g)])
                        emit_early(itq, psA)

                    proj_half("wq", QT, qch, 0, HS)
                    proj_half("wk", KT, kch, 0, HS)
                    emit_early(eA, psA)
                    proj_half("wq", QT, qch, HS, S, interleave=eB)
                    proj_half("wk", KT, kch, HS, S)

                    # xv reuses the q-chunk slots; issue its DMAs only now
                    # that every qch reader is emitted (WAR ordering).
                    vch = mk_chunks("v")
                    for e in range(EC):
                        nc.sync.dma_start(out=vch[e],
                                          in_=d_xv[e * P:(e + 1) * P, :])
                    for p in range(HL // 2):
                        nc.sync.dma_start(
                            out=wo_sb[p],
                            in_=d_wo[p * 2 * D:(p + 1) * 2 * D, :])

                    emit_early(eC, psA)

                    # V projection: 4 passes of 4 s-tiles, e-OUTER within a
                    # pass so the first pass streams JIT with the arriving
                    # xv chunks instead of waiting for the whole tensor.
                    for vp in range(NST // 4):
                        sts = range(4 * vp, 4 * vp + 4)
                        pss = {st: psA.tile([P, VW], f32,
                                            tag=f"pj{(st % 4) // 2}{st % 2}",
                                            name=f"psv{st}") for st in sts}
                        for e in range(EC):
                            for st in sts:
                                nc.tensor.matmul(
                                    pss[st],
                                    lhsT=vch[e][:, st * P:(st + 1) * P],
                                    rhs=w_sb["wv"][:, e, :],
                                    start=(e == 0), stop=(e == EC - 1))
                        for st in sts:
                            nc.vector.tensor_copy(V[st], pss[st])
                            onescols = V[st].rearrange(
                                "p (h c) -> p h c", c=D1)[:, :, D]
                            nc.gpsimd.memset(onescols, 1.0)

                # ---------------- phase B: attention ----------------
                with tc.tile_pool(name="psS", bufs=2, space="PSUM") as psS, \
                     tc.tile_pool(name="psPV", bufs=2, space="PSUM") as psPV, \
                     tc.tile_pool(name="psX", bufs=2, space="PSUM") as psX:

                    heads_done = {g: 0 for g in range(NQG)}
                    attnN = {}   # (g, p, j) -> sbuf tile [P, 2D]
                    # global FIFO of deferred work (PV matmuls, finishes,
                    # transposes, output projections), drained N jobs behind
                    # the eagerly-emitted scores/exp stream.
                    jobs = []
                    # live probs-ring tiles: pre-emitted pbs count at B start;
                    # each pv job emission frees one slot (emission order is
                    # what matters for deadlock-freedom)
                    alive = [sum(len(v) for v in early_pend.values())]
                    _pb_cap = int(os.environ.get("K2_PROBS", "40")) - 2

                    def drain(limit):
                        while len(jobs) > limit:
                            jobs.pop(0)()

                    def make_pv_job(pair, pb, g, h, pvq, state, total):
                        def run():
                            alive[0] -= 1
                            # ascending kt within the pair.  PSUM start=True
                            # zeroes the whole 2KB zero-region (bank), so
                            # only the very FIRST matmul into this tile may
                            # carry start=True; later first-touches of other
                            # j-regions overwrite via the pending-zero bits.
                            for x, kt in sorted(enumerate(pair),
                                                key=lambda t: t[1]):
                                for j in range(JB):
                                    if not block_live[kt, g * JB + j]:
                                        continue
                                    state["n"] += 1
                                    nc.tensor.matmul(
                                        pvq[:, j * D1:(j + 1) * D1],
                                        lhsT=pb[:, x * QG + j * P:
                                                x * QG + (j + 1) * P],
                                        rhs=V[kt][:, h * D1:(h + 1) * D1],
                                        start=(state["n"] == 1),
                                        stop=(state["n"] == total),
                                        skip_group_check=True)
                        return run

                    def make_hfin_job(g, h, pvq):
                        p, po = h // 2, (h % 2) * D

                        def run():
                            # per q-block: f32 reciprocal of the Z column,
                            # then a per-partition scaled copy into the
                            # head-pair attnN tile (bf16).  For the closing
                            # groups the odd head's copy goes to ACT so both
                            # halves land in parallel.
                            for j in range(JB):
                                if h % 2 == 0:
                                    attnN[(g, p, j)] = attnN_pool.tile(
                                        [P, 2 * D], bf16, tag="an",
                                        name="an")
                                an = attnN[(g, p, j)]
                                zt = z_pool.tile([P, 1], f32, tag="zt",
                                                 name="zt")
                                nc.vector.reciprocal(
                                    zt, pvq[:, j * D1 + D:(j + 1) * D1])
                                nc.vector.tensor_scalar_mul(
                                    an[:, po:po + D],
                                    pvq[:, j * D1:j * D1 + D], zt)
                        return run

                    def make_tr_job(g, p):
                        def run():
                            # transpose the head-pair q-blocks to [2d, q]
                            # (identity matmuls) landing side-by-side in ONE
                            # psum tile, then a single [P, QG] copy to sbuf
                            pst = psX.tile([P, QG], bf16, tag="x",
                                           name="pst")
                            for j in range(JB):
                                nc.tensor.transpose(
                                    pst[:, j * P:(j + 1) * P],
                                    attnN.pop((g, p, j)), ident)
                            nc.vector.tensor_copy(attnP[p][g], pst)
                        return run

                    def make_gfin_job(g, j):
                        def run():
                            # ---- output projection for q-block j of g ----
                            st = g * JB + j
                            off = j * P
                            ot = out_pool.tile([P, NEG * EGW], bf16,
                                               tag="ot", name="ot")
                            for eg in range(NEG):
                                ops = psX.tile([P, EGW], f32, tag="x",
                                               name="opso")
                                for p in range(HL // 2):
                                    nc.tensor.matmul(
                                        ops,
                                        lhsT=attnP[p][g][:, off:off + P],
                                        rhs=wo_sb[p][:, eg * EGW:
                                                     (eg + 1) * EGW],
                                        start=(p == 0),
                                        stop=(p == HL // 2 - 1))
                                # alternate the copies across ACT and DVE
                                # for the closing groups so the drain chain
                                # is halved; the rest stay on DVE
                                if g in tail_gfin and eg == 0:
                                    nc.scalar.copy(
                                        ot[:, eg * EGW:(eg + 1) * EGW],
                                        ops)
                                else:
                                    nc.vector.tensor_copy(
                                        ot[:, eg * EGW:(eg + 1) * EGW],
                                        ops)
                                if g in tail_gfin:
                                    # per-eg DMA so the close-out chain is
                                    # copy->small-DMA instead of full-row
                                    nc.sync.dma_start(
                                        out=d_out[st * P:(st + 1) * P,
                                                  eg * EGW:(eg + 1) * EGW],
                                        in_=ot[:, eg * EGW:(eg + 1) * EGW])
                            if g not in tail_gfin:
                                nc.sync.dma_start(
                                    out=d_out[st * P:(st + 1) * P, :],
                                    in_=ot)
                        return run

                    for g, h in work:
                        pvq = psPV.tile([P, JB * D1], f32, tag="pv",
                                        name="pvq")

                        state = {"n": 0}
                        total = sum(len(live_kts(g, j)) for j in range(JB))
                        pend = early_pend.pop((g, h), None)
                        if pend is None:
                            for pair in group_pairs(g):
                                while alive[0] >= _pb_cap and jobs:
                                    jobs.pop(0)()
                                alive[0] += 1
                                pb = emit_pair(g, h, pair, psS, "s",
                                               mask_dve=g in (ga, gb))
                                jobs.append(make_pv_job(pair, pb, g, h, pvq,
                                                        state, total))
                                drain(_drain_n)
                        else:
                            for pair, pb in pend:
                                jobs.append(make_pv_job(pair, pb, g, h, pvq,
                                                        state, total))
                                drain(int(os.environ.get("K2_DRAIN_E", "48")))

                        jobs.append(make_hfin_job(g, h, pvq))
                        if h % 2 == 1:
                            jobs.append(make_tr_job(g, h // 2))
                        heads_done[g] += 1
                        if heads_done[g] == HL:
                            for j in range(JB):
                                jobs.append(make_gfin_job(g, j))
                        drain(_drain_n + 1)

                    drain(0)

        for _rep in range(repeat):
            emit_once()

    _split_multi_waits(nc)
    return nc


# ---------------------------------------------------------------------------
# Host entry point
# ---------------------------------------------------------------------------
LAST_EXEC_NS = None
LAST_RESULT = None


def kernel(query, key, value, mask, Wq, Wk, Wv, Wo, bo):
    global LAST_EXEC_NS, LAST_RESULT
    _install_tile_drain_patch()
    from concourse.bass_utils import run_bass_kernel_spmd

    B, S, E = 2, 2048, 1024
    H, D = 16, 64
    N_CORES = 8
    BG = 2                    # batch groups
    HG = N_CORES // BG        # head groups per batch
    HL = H // HG              # heads per core
    DIM = HL * D

    query = np.asarray(query, dtype=np.float32)
    key = np.asarray(key, dtype=np.float32)
    value = np.asarray(value, dtype=np.float32)
    mask2d = np.asarray(mask).reshape(S, S).astype(bool)
    Wq = np.asarray(Wq, dtype=np.float32)
    Wk = np.asarray(Wk, dtype=np.float32)
    Wv = np.asarray(Wv, dtype=np.float32)
    Wo = np.asarray(Wo, dtype=np.float32)
    bo = np.asarray(bo, dtype=np.float32)

    bias_idx, biases, block_live = classify_mask(mask2d, S)
    nuniq = len(biases)
    bias_stack = (np.concatenate(biases, axis=1) if nuniq
                  else np.zeros((128, 128), np.float32))

    nc = build_nc(S, E, D, HL, bias_idx, block_live, nuniq)

    scale = np.float32(1.0 / np.sqrt(D))
    in_maps = []
    for c in range(N_CORES):
        b, hg = c // HG, c % HG
        cols = slice(hg * DIM, (hg + 1) * DIM)
        wv_l = Wv[:, cols].reshape(E, HL, D)
        wv_aug = np.zeros((E, HL, D + 1), np.float32)
        wv_aug[:, :, :D] = wv_l
        in_maps.append({
            "xqT": _bf16(query[b].T),
            "xkT": _bf16(key[b].T),
            "xvT": _bf16(value[b].T),
            "wq": _bf16(Wq[:, cols] * scale),
            "wk": _bf16(Wk[:, cols]),
            "wv": _bf16(wv_aug.reshape(E, HL * (D + 1))),
            "wo": _bf16(Wo[cols, :]),
            "biasT": _bf16(bias_stack),
        })

    res = run_bass_kernel_spmd(nc, in_maps, list(range(N_CORES)))
    LAST_RESULT = res
    LAST_EXEC_NS = res.exec_time_ns or res.mean_exec_time_ns

    out = np.empty((B, S, E), np.float32)
    for b in range(BG):
        acc = res.results[b * HG]["out_p"].astype(np.float32)
        for j in range(1, HG):
            acc = acc + res.results[b * HG + j]["out_p"]
        out[b] = acc + bo[None, :]
    return out


def _bf16(a):
    import ml_dtypes
    return np.ascontiguousarray(np.asarray(a, np.float32)).astype(
        ml_dtypes.bfloat16)


# revision 61
# speedup vs baseline: 1.1921x; 1.0197x over previous
"""Multi-head attention (B=2, S=2048, E=1024, H=16) on 8 Trainium2 NeuronCores.

Sharding: core c -> batch c//4, heads 4*(c%4)..4*(c%4)+3  (data + head parallel).
Each core computes a partial output projection [S, E] over its 256 head-dims;
the host sums the 4 bf16 partials per batch in f32 and adds the output bias
(the "all-reduce" happens in the unshard step).

On-chip layouts (contraction always on the partition dim; host pre-transposes
query/key/value):
  QT, KT  [dim, S]   = Wx^T @ X^T      (rhs = X^T chunks streamed from HBM)
  V       [S, dim+ones]                 (natural; a ones column per head makes
                                         the PV matmul also emit softmax sums)
  scoresT [keys, q]  = KT_tile^T-block @ QT     per (head, q-group, key-tile),
                       column-trimmed to the causally live range
  probsT  = exp(scoresT) (bf16)         unnormalized; diagonal blocks masked by
                                        a post-exp 0/1 multiply (Pool/DVE);
                                        fully-dead blocks are simply never read
  pvq     [q, 4*(d+1)] = probs-block^T-stationary @ (V|1)   per 128-q block,
                       accumulated over key tiles; col d=64 is Z = sum of probs.
                       Probs-stationary puts the SMALL dim (65) on the moving
                       free axis: 35.4k PE columns vs 69.6k the other way.
                       All 4 q-block groups share one PSUM bank, so only the
                       very first matmul carries start=True (start zeroes the
                       whole 2KB zero-region; later first-touches overwrite
                       via the pending-zero bits).
  zinv    [q, 4]     = one strided f32 DVE reciprocal over the 4 Z columns
  attnN   [q, 2d]    = pvq * zinv (DVE tensor_scalar, head pair packed)
  attnP   [2d, q]    = 4 PE transposes (identity matmul) landing side-by-side
                       in one PSUM tile + a single [128, 512] copy to sbuf
  partial [S, E]     = attnP^T-chunks @ Wo-rows (bf16 out, host-summed)

Matmul operands are bf16 (full PE rate); all accumulation is fp32 in PSUM.
fp8 DoubleRow was evaluated and rejected: e4m3 quantization of X/W/V/attn
each push the absmax-rel error past the 2e-2 gate (host emulation).

Cost-model notes (TimelineSim): matmul cost = out-free-columns x pe_cycle,
independent of K and M, Ldweights free; DMA is a single serialized engine
(~52us of transfers) with ~625ns/DMA HWDGE pacing, so DMA count is capped
and issue order == transfer order.

Schedule (~120.4us; PE busy ~100.5us of ~240.7k columns):
  Phase A is COLUMN-PHASED: Q/K are projected for query columns [0:1024)
  (both m-tiles, 4 psum banks, JIT against the half-chunk stream), then
  columns [1024:2048).  Scores+exp for low-column items are pre-emitted in
  three waves so the ACT exp pipe starts ~15us in: eA (g0 h0-2) after the
  first half, eB (g0h3 + all g1) interleaved into the e-steps of the
  second-half Q projection, eC (g3 h0-1) before the V projection.  V runs
  in 4 passes of 4 s-tiles (e-outer) so pass 0 streams JIT with xv.
  Phase B walks the items largest-first with a global FIFO of deferred
  jobs (PV, finish, transpose, output projection) drained K2_DRAIN=64 jobs
  behind the eager scores/exp stream; a probs-ring pressure guard force-
  drains PV jobs before the 40-tile pool can cycle-deadlock.  The output
  projection is per-q-block with ONE output DMA each (the kernel end is
  HWDGE-paced at ~625ns/DMA, so fewer, larger transfers drain faster); the
  two closing groups split their psum copies across ACT+DVE; the tail is
  anchored by g0h3 (PV-only) sandwiched in g2's close; the closing groups'
  second output-projection psum tiles borrow the then-idle score-psum ring
  (4-deep ring at the tail).
"""

import sys

for _p in ("/opt/trn_rl_repo", "/root/.axon_site/_ro/trn_rl_repo"):
    if _p not in sys.path:
        sys.path.insert(0, _p)

import numpy as np


# ---------------------------------------------------------------------------
# Patch: the walrus build in this container rejects >1 sem wait on one CTRL
# instruction ("Too many sync wait commands") and the TileContext exit drain
# aggregates every outstanding proc's wait onto a single Drain. Spill the
# excess waits onto SP nops (1 wait each) emitted right after the drain.
# ---------------------------------------------------------------------------
def _install_tile_drain_patch():
    import concourse.tile as tile
    import concourse.mybir as mybir
    from concourse.vector_clock import ScopedClock

    if getattr(tile.TileContext, "_drain_patch_installed", False):
        return

    def _patched_drain_and_barrier(self, tick_clock, wait_clock):
        drain_inst = self.nc.sync.drain()
        wait_clock.add_sem_waits(
            drain_inst.ins, ScopedClock({None: tick_clock.global_clock})
        )
        si = drain_inst.ins.sync_info
        waits = list(si.on_wait) if si and si.on_wait else []
        if len(waits) > 1:
            si.on_wait = waits[:1]
            for w in waits[1:]:
                nop = self.nc.sync.nop(nofuse=True, hint="drain_wait_spill")
                nop.ins.sync_info = mybir.SyncInfo(on_wait=[w], on_update=[])
        self.nc.all_engine_barrier()
        assert self.sems is not None
        popped = self.nc._tile_sem_poison_stack.pop()
        assert popped is self._sem_poison
        self.nc.clear_and_free_semaphores(list(self.sems.allocated().values()))
        self.nc.all_engine_barrier()

    tile.TileContext._drain_and_barrier = _patched_drain_and_barrier
    tile.TileContext._drain_patch_installed = True


def _split_multi_waits(nc, maxw=1):
    """Walrus here allows only `maxw` sem-wait commands per instruction.
    Hoist excess waits onto engine-queue NoOps inserted just before the
    instruction (the sequencer executes them in order, so semantics are
    identical)."""
    import concourse.mybir as mybir

    ctr = 0
    for bb in nc.main_func.blocks:
        new = []
        for inst in bb.instructions:
            si = inst.sync_info
            waits = list(si.on_wait) if si and si.on_wait else []
            if len(waits) > maxw:
                extras = waits[:-maxw]
                si.on_wait = waits[-maxw:]
                for i in range(0, len(extras), maxw):
                    nop = mybir.InstNoOp(
                        name=f"I-waitspill-{ctr}", engine=inst.engine,
                        ins=[], outs=[])
                    ctr += 1
                    nop.sync_info = mybir.SyncInfo(
                        on_wait=extras[i:i + maxw], on_update=[])
                    try:
                        nc.register_instruction(nop, overwrite=True)
                    except Exception:
                        pass
                    new.append(nop)
            new.append(inst)
        bb.instructions = new


# ---------------------------------------------------------------------------
# Mask classification (host side, from the actual mask array).
# Blocks are 128x128 in the *transposed* score layout: block (kt, qb) covers
# keys kt*128.. x queries qb*128... Returns per-block bias indices into a
# stack of unique multiplicative 0/1 mask blocks.
# ---------------------------------------------------------------------------
def classify_mask(mask2d, S, KB=128):
    nb = S // KB
    assert mask2d.shape == (S, S)
    assert mask2d.any(axis=1).all(), "a query row with no attended key"
    maskT = mask2d.T  # [keys, q]
    uniq = {}
    biases = []
    bias_idx = {}  # (kt, qb) -> None (all attended) or index
    block_live = np.zeros((nb, nb), dtype=bool)  # any attended key in block
    for kt in range(nb):
        for qb in range(nb):
            blk = maskT[kt * KB:(kt + 1) * KB, qb * KB:(qb + 1) * KB]
            if blk.all():
                bias_idx[(kt, qb)] = None
                block_live[kt, qb] = True
            else:
                b = np.where(blk, np.float32(1.0), np.float32(0.0))
                key = b.tobytes()
                if key not in uniq:
                    uniq[key] = len(biases)
                    biases.append(b)
                bias_idx[(kt, qb)] = uniq[key]
                block_live[kt, qb] = blk.any()
    return bias_idx, biases, block_live


# ---------------------------------------------------------------------------
# Bass program builder (one SPMD program, same for all cores).
# ---------------------------------------------------------------------------
def build_nc(S, E, D, HL, bias_idx, block_live, nuniq, repeat=1):
    import concourse.bass as bass
    import concourse.mybir as mybir
    import concourse.tile as tile
    from concourse.masks import make_identity

    f32 = mybir.dt.float32
    bf16 = mybir.dt.bfloat16
    Act = mybir.ActivationFunctionType

    P = 128
    EC = E // P              # E chunks (contraction tiles for projections)
    DIM = HL * D             # this core's head dims (256)
    MT = DIM // P            # m-tiles of QT/KT (2)
    QG = 512                 # q-group width
    JB = QG // P             # 128-q blocks per group (4)
    NQG = S // QG
    NKT = S // P             # key tiles
    NST = S // P             # s tiles
    D1 = D + 1
    VW = HL * D1             # V width incl. ones columns (260)
    EGW = min(QG, E)         # output E slice width
    NEG = E // EGW           # output E slices (2)

    # key tiles needed per q-group
    def kts_for_group(g):
        out = []
        for kt in range(NKT):
            if any(block_live[kt, g * JB + j] for j in range(JB)):
                out.append(kt)
        return out

    # first live column (within the group's QG window) for a key tile
    def qstart(kt, g):
        for j in range(JB):
            if block_live[kt, g * JB + j]:
                return j * P
        return QG

    def live_kts(g, j):
        return [kt for kt in kts_for_group(g) if block_live[kt, g * JB + j]]

    nc = bass.Bass()
    dp = nc.declare_dram_parameter
    d_xq = dp("xqT", [E, S], bf16, isOutput=False)
    d_xk = dp("xkT", [E, S], bf16, isOutput=False)
    d_xv = dp("xvT", [E, S], bf16, isOutput=False)
    d_wq = dp("wq", [E, DIM], bf16, isOutput=False)
    d_wk = dp("wk", [E, DIM], bf16, isOutput=False)
    d_wv = dp("wv", [E, VW], bf16, isOutput=False)
    d_wo = dp("wo", [DIM, E], bf16, isOutput=False)
    d_bias = dp("biasT", [P, max(nuniq, 1) * P], bf16, isOutput=False)
    d_out = dp("out_p", [S, E], bf16, isOutput=True)

    import os
    import contextlib
    with tile.TileContext(nc) as tc, contextlib.ExitStack() as _stk:
        consts = _stk.enter_context(tc.tile_pool(name="consts", bufs=1))

        # weight tiles: [E, n] rearranged so chunk e lives at w_sb[:, e, :].
        w_sb = {}
        for nm, width in (("wq", DIM), ("wk", DIM), ("wv", VW)):
            w_sb[nm] = consts.tile([P, EC, width], bf16, name=f"sb_{nm}",
                                   tag=f"sb_{nm}")
        w_dram = {"wq": d_wq, "wk": d_wk, "wv": d_wv}
        wo_sb = [consts.tile([2 * D, E], bf16, name=f"sb_wo{p}",
                             tag=f"sb_wo{p}") for p in range(HL // 2)]
        bias_sb = consts.tile([P, max(nuniq, 1) * P], bf16, name="sb_bias")
        ident = consts.tile([P, P], bf16, name="ident")
        make_identity(nc, ident)

        def load_w(nm, lo=0, hi=None):
            hi = EC if hi is None else hi
            src = w_dram[nm][:, :].rearrange("(e p) n -> p e n", p=P)
            nc.sync.dma_start(out=w_sb[nm][:, lo:hi, :], in_=src[:, lo:hi, :])

        def emit_once():
            # persistent projection outputs
            QT = [consts.tile([P, S], bf16, name=f"QT{m}", tag=f"QT{m}")
                  for m in range(MT)]
            KT = [consts.tile([P, S], bf16, name=f"KT{m}", tag=f"KT{m}")
                  for m in range(MT)]
            V = [consts.tile([P, VW], bf16, name=f"V{s}", tag=f"V{s}")
                 for s in range(NST)]
            # attnT stored as head-PAIR tiles [128, QG]: head 2p ->
            # partitions 0..63, head 2p+1 -> 64..127, so the output
            # projection contracts K=128.
            attnP = [[consts.tile([2 * D, QG], bf16, name=f"attnP{p}g{g}",
                                  tag=f"attnP{p}g{g}") for g in range(NQG)]
                     for p in range(HL // 2)]

            g_order = sorted(range(NQG),
                             key=lambda g: -len(kts_for_group(g)))
            _drain_n = int(os.environ.get("K2_DRAIN", "64"))
            # g0_ = smallest group ... g3_ = largest
            g0_, g1_, g2_, g3_ = (g_order[3], g_order[2], g_order[1],
                                  g_order[0])
            # Items pre-emitted (scores+exp) during phase A, in three waves:
            #   eA: right after the first column-half of Q/K projections
            #       (these only touch QT/KT columns < S/2)
            #   eB: between the second-half Q and K projections (ditto)
            #   eC: after full projections, filling the PE while xv streams
            eA = [(g0_, 0), (g0_, 1), (g0_, 2)]
            eB = [(g0_, 3), (g1_, 0), (g1_, 1), (g1_, 2), (g1_, 3)]
            eC = [(g3_, 0), (g3_, 1)]
            early = eA + eB + eC
            # B-phase processing order: early items' PV first (consumes the
            # held probs, frees the ring), then the B-scored items; the
            # smallest group's last head anchors a short tail.
            # B-scored items (g3 h2/h3, g2 all) spread one-per-two slots so
            # the exp stream never dries; pre-emitted PV items fill between.
            work = [(g0_, 0), (g0_, 1), (g0_, 2), (g1_, 0), (g1_, 1),
                    (g1_, 2), (g3_, 0), (g3_, 1), (g1_, 3),
                    (g3_, 2), (g2_, 0), (g3_, 3), (g2_, 1),
                    (g0_, 3), (g2_, 2), (g2_, 3)]
            assert sorted(work) == sorted(
                (g, h) for g in range(NQG) for h in range(HL))
            tail_gfin = {g for g, h in work[-3:]}
            last_g = work[-1][0]
            ga, gb = g1_, g0_   # groups whose mask-muls go to DVE

            def group_pairs(g):
                kts = kts_for_group(g)
                prs = [kts[i:i + 2] for i in range(0, len(kts), 2)]
                # within a pair, larger qstart first: one exp op then covers
                # [qstart(first) : end] only
                return [sorted(p, key=lambda kt: -qstart(kt, g)) for p in prs]

            with tc.tile_pool(name="probs", bufs=int(os.environ.get("K2_PROBS", "40"))) as probs_pool, \
                 tc.tile_pool(name="attnN", bufs=int(os.environ.get("K2_ATTN", "12"))) as attnN_pool, \
                 tc.tile_pool(name="zrow", bufs=8) as z_pool, \
                 tc.tile_pool(name="outst", bufs=4) as out_pool:

                def emit_pair(g, h, pair, pool, tag, mask_dve=False):
                    """scores + exp + diagonal masking for one key-tile pair"""
                    m, po = h // 2, (h % 2) * D
                    w = len(pair) * QG
                    s0 = qstart(pair[0], g)
                    sps = pool.tile([P, 2 * QG], f32, tag=tag, bufs=2,
                                    name="sps")
                    pb = probs_pool.tile([P, 2 * QG], bf16, tag="pb",
                                         name="pb")
                    for x, kt in enumerate(pair):
                        qs = qstart(kt, g)
                        nc.tensor.matmul(
                            sps[:, x * QG + qs:(x + 1) * QG],
                            lhsT=KT[m][po:po + D, kt * P:(kt + 1) * P],
                            rhs=QT[m][po:po + D, g * QG + qs:(g + 1) * QG],
                            start=True, stop=True)
                    qs1 = qstart(pair[1], g) if len(pair) > 1 else 0
                    if len(pair) > 1 and qs1 > 0:
                        # skip the dead hole [QG : QG+qs1] (never written)
                        nc.scalar.activation(pb[:, s0:QG], sps[:, s0:QG],
                                             Act.Exp)
                        nc.scalar.activation(pb[:, QG + qs1:w],
                                             sps[:, QG + qs1:w], Act.Exp)
                    else:
                        nc.scalar.activation(pb[:, s0:w], sps[:, s0:w],
                                             Act.Exp)
                    # masking applied AFTER exp (multiplicative 0/1, exact)
                    # on partially-live (diagonal) blocks only; fully-dead
                    # blocks are never read by the PV matmuls.
                    for x, kt in enumerate(pair):
                        for j in range(JB):
                            qb = g * JB + j
                            bidx = bias_idx[(kt, qb)]
                            if bidx is None or not block_live[kt, qb]:
                                continue
                            blkslice = pb[:, x * QG + j * P:
                                          x * QG + (j + 1) * P]
                            if mask_dve:
                                nc.vector.tensor_mul(
                                    blkslice, blkslice,
                                    bias_sb[:, bidx * P:(bidx + 1) * P])
                            else:
                                nc.gpsimd.tensor_mul(
                                    blkslice, blkslice,
                                    bias_sb[:, bidx * P:(bidx + 1) * P])
                    return pb

                # ---------------- phase A: projections ----------------
                # Column-phased: Q/K are projected for the first S/2 query
                # columns (both m-tiles) as soon as those half-chunks land,
                # so scores+exp for the low-column groups start ~15us in.
                HS = S // 2
                early_pend = {}

                def emit_early(items, pool):
                    for g, h in items:
                        early_pend[(g, h)] = [
                            (pair, emit_pair(g, h, pair, pool, "se"))
                            for pair in group_pairs(g)]

                with tc.tile_pool(name="xt", bufs=2 * EC) as xt_pool, \
                     tc.tile_pool(name="psA", bufs=1, space="PSUM") as psA:

                    def mk_chunks(nm):
                        return [xt_pool.tile([P, S], bf16, tag="xt",
                                             name=f"{nm}{e}")
                                for e in range(EC)]

                    qch, kch = mk_chunks("q"), mk_chunks("k")

                    def dma_half(dram, chunks, lo, hi, skip_e0=False):
                        for e in range(1 if skip_e0 else 0, EC):
                            nc.sync.dma_start(
                                out=chunks[e][:, lo:hi],
                                in_=dram[e * P:(e + 1) * P, lo:hi])

                    # DMA issue order = transfer order (single DMA engine):
                    # minimal prefix first so the first matmul starts ~2.5us.
                    load_w("wq", 0, 1)
                    nc.sync.dma_start(out=qch[0][:, 0:HS],
                                      in_=d_xq[0:P, 0:HS])
                    nc.sync.dma_start(out=qch[1][:, 0:HS],
                                      in_=d_xq[P:2 * P, 0:HS])
                    load_w("wq", 1, EC)
                    for e in range(2, EC):
                        nc.sync.dma_start(out=qch[e][:, 0:HS],
                                          in_=d_xq[e * P:(e + 1) * P, 0:HS])
                    load_w("wk")
                    # the early waves' diagonal mask-muls read bias_sb, so
                    # its DMA must be issued before they are emitted
                    nc.sync.dma_start(out=bias_sb, in_=d_bias[:, :])
                    dma_half(d_xk, kch, 0, HS)
                    dma_half(d_xq, qch, HS, S)
                    dma_half(d_xk, kch, HS, S)
                    load_w("wv")

                    def proj_half(wname, dst, chunks, c0, c1,
                                  interleave=()):
                        gs = list(range(c0 // QG, c1 // QG))
                        itq = list(interleave)
                        pss = {}
                        for m in range(MT):
                            for g in gs:
                                pss[(m, g)] = psA.tile(
                                    [P, QG], f32, tag=f"pj{m}{g % 2}",
                                    name="pspj")
                        for e in range(EC):
                            for m in range(MT):
                                for g in gs:
                                    nc.tensor.matmul(
                                        pss[(m, g)],
                                        lhsT=w_sb[wname][:, e,
                                                         m * P:(m + 1) * P],
                                        rhs=chunks[e][:, g * QG:(g + 1) * QG],
                                        start=(e == 0), stop=(e == EC - 1))
                            # pre-emitted scores between e-steps keep the
                            # exp stream fed while this projection runs
                            if e % 2 == 1 and itq:
                                emit_early([itq.pop(0)], psA)
                        # score-relevant (low-g) copies first so the next
                        # early wave's scores unblock as soon as possible
                        for g in gs:
                            for m in range(MT):
                                nc.vector.tensor_copy(
                                    dst[m][:, g * QG:(g + 1) * QG],
                                    pss[(m, g)])
                        emit_early(itq, psA)

                    # warm the PE p-state during the DMA-wait head: the
                    # cost model halves matmul speed for the first ~3us of
                    # PE activity; burn that on dummy identity matmuls
                    # before the first real projection matmul arrives.
                    wup = psA.tile([P, P], f32, tag="pj00", name="wup")
                    for wi in range(24):
                        nc.tensor.matmul(wup, lhsT=ident, rhs=ident,
                                         start=True, stop=True)

                    proj_half("wq", QT, qch, 0, HS)
                    proj_half("wk", KT, kch, 0, HS)
                    emit_early(eA, psA)
                    proj_half("wq", QT, qch, HS, S, interleave=eB)
                    proj_half("wk", KT, kch, HS, S)

                    # xv reuses the q-chunk slots; issue its DMAs only now
                    # that every qch reader is emitted (WAR ordering).
                    vch = mk_chunks("v")
                    for e in range(EC):
                        nc.sync.dma_start(out=vch[e],
                                          in_=d_xv[e * P:(e + 1) * P, :])
                    for p in range(HL // 2):
                        nc.sync.dma_start(
                            out=wo_sb[p],
                            in_=d_wo[p * 2 * D:(p + 1) * 2 * D, :])

                    emit_early(eC, psA)

                    # V projection: 4 passes of 4 s-tiles, e-OUTER within a
                    # pass so the first pass streams JIT with the arriving
                    # xv chunks instead of waiting for the whole tensor.
                    for vp in range(NST // 4):
                        sts = range(4 * vp, 4 * vp + 4)
                        pss = {st: psA.tile([P, VW], f32,
                                            tag=f"pj{(st % 4) // 2}{st % 2}",
                                            name=f"psv{st}") for st in sts}
                        for e in range(EC):
                            for st in sts:
                                nc.tensor.matmul(
                                    pss[st],
                                    lhsT=vch[e][:, st * P:(st + 1) * P],
                                    rhs=w_sb["wv"][:, e, :],
                                    start=(e == 0), stop=(e == EC - 1))
                        for st in sts:
                            # late passes' copies go to ACT so the DVE queue
                            # reaches the first finish jobs sooner (V[8:] is
                            # only needed by the large groups' PV, later)
                            if st >= 8:
                                nc.scalar.copy(V[st], pss[st])
                            else:
                                nc.vector.tensor_copy(V[st], pss[st])
                            onescols = V[st].rearrange(
                                "p (h c) -> p h c", c=D1)[:, :, D]
                            nc.gpsimd.memset(onescols, 1.0)

                # ---------------- phase B: attention ----------------
                with tc.tile_pool(name="psS", bufs=2, space="PSUM") as psS, \
                     tc.tile_pool(name="psPV", bufs=2, space="PSUM") as psPV, \
                     tc.tile_pool(name="psX", bufs=2, space="PSUM") as psX:

                    heads_done = {g: 0 for g in range(NQG)}
                    attnN = {}   # (g, p, j) -> sbuf tile [P, 2D]
                    # global FIFO of deferred work (PV matmuls, finishes,
                    # transposes, output projections), drained N jobs behind
                    # the eagerly-emitted scores/exp stream.
                    jobs = []
                    # live probs-ring tiles: pre-emitted pbs count at B start;
                    # each pv job emission frees one slot (emission order is
                    # what matters for deadlock-freedom)
                    alive = [sum(len(v) for v in early_pend.values())]
                    _pb_cap = int(os.environ.get("K2_PROBS", "40")) - 2

                    def drain(limit):
                        while len(jobs) > limit:
                            jobs.pop(0)()

                    def make_pv_job(pair, pb, g, h, pvq, state, total):
                        def run():
                            alive[0] -= 1
                            # ascending kt within the pair.  PSUM start=True
                            # zeroes the whole 2KB zero-region (bank), so
                            # only the very FIRST matmul into this tile may
                            # carry start=True; later first-touches of other
                            # j-regions overwrite via the pending-zero bits.
                            for x, kt in sorted(enumerate(pair),
                                                key=lambda t: t[1]):
                                for j in range(JB):
                                    if not block_live[kt, g * JB + j]:
                                        continue
                                    state["n"] += 1
                                    nc.tensor.matmul(
                                        pvq[:, j * D1:(j + 1) * D1],
                                        lhsT=pb[:, x * QG + j * P:
                                                x * QG + (j + 1) * P],
                                        rhs=V[kt][:, h * D1:(h + 1) * D1],
                                        start=(state["n"] == 1),
                                        stop=(state["n"] == total),
                                        skip_group_check=True)
                        return run

                    def make_hfin_job(g, h, pvq):
                        p, po = h // 2, (h % 2) * D

                        def run():
                            # per q-block: f32 reciprocal of the Z column,
                            # then a per-partition scaled copy into the
                            # head-pair attnN tile (bf16).  For the closing
                            # groups the odd head's copy goes to ACT so both
                            # halves land in parallel.
                            for j in range(JB):
                                if h % 2 == 0:
                                    attnN[(g, p, j)] = attnN_pool.tile(
                                        [P, 2 * D], bf16, tag="an",
                                        name="an")
                                an = attnN[(g, p, j)]
                                zt = z_pool.tile([P, 1], f32, tag="zt",
                                                 name="zt")
                                nc.vector.reciprocal(
                                    zt, pvq[:, j * D1 + D:(j + 1) * D1])
                                nc.vector.tensor_scalar_mul(
                                    an[:, po:po + D],
                                    pvq[:, j * D1:j * D1 + D], zt)
                        return run

                    def make_tr_job(g, p):
                        def run():
                            # transpose the head-pair q-blocks to [2d, q]
                            # (identity matmuls) landing side-by-side in ONE
                            # psum tile, then a single [P, QG] copy to sbuf
                            pst = psX.tile([P, QG], bf16, tag="x",
                                           name="pst")
                            for j in range(JB):
                                nc.tensor.transpose(
                                    pst[:, j * P:(j + 1) * P],
                                    attnN.pop((g, p, j)), ident)
                            nc.vector.tensor_copy(attnP[p][g], pst)
                        return run

                    def make_gfin_job(g, j):
                        def run():
                            # ---- output projection for q-block j of g ----
                            st = g * JB + j
                            off = j * P
                            ot = out_pool.tile([P, NEG * EGW], bf16,
                                               tag="ot", name="ot")
                            for eg in range(NEG):
                                # the closing groups borrow the (idle) score
                                # psum ring for every other tile: 4-deep ring
                                if g in tail_gfin and eg == 1:
                                    ops = psS.tile([P, EGW], f32, tag="s",
                                                   name="opso", bufs=2)
                                else:
                                    ops = psX.tile([P, EGW], f32, tag="x",
                                                   name="opso")
                                for p in range(HL // 2):
                                    nc.tensor.matmul(
                                        ops,
                                        lhsT=attnP[p][g][:, off:off + P],
                                        rhs=wo_sb[p][:, eg * EGW:
                                                     (eg + 1) * EGW],
                                        start=(p == 0),
                                        stop=(p == HL // 2 - 1))
                                # alternate the copies across ACT and DVE
                                # for the closing groups so the drain chain
                                # is halved; the rest stay on DVE
                                if g in tail_gfin and eg == 0:
                                    nc.scalar.copy(
                                        ot[:, eg * EGW:(eg + 1) * EGW],
                                        ops)
                                else:
                                    nc.vector.tensor_copy(
                                        ot[:, eg * EGW:(eg + 1) * EGW],
                                        ops)
                            # one DMA per q-block: the end of the kernel is
                            # HWDGE-paced (~625ns per DMA), so fewer, larger
                            # transfers drain faster than per-eg splits
                            nc.sync.dma_start(
                                out=d_out[st * P:(st + 1) * P, :], in_=ot)
                        return run

                    for g, h in work:
                        pvq = psPV.tile([P, JB * D1], f32, tag="pv",
                                        name="pvq")

                        state = {"n": 0}
                        total = sum(len(live_kts(g, j)) for j in range(JB))
                        pend = early_pend.pop((g, h), None)
                        if pend is None:
                            for pair in group_pairs(g):
                                while alive[0] >= _pb_cap and jobs:
                                    jobs.pop(0)()
                                alive[0] += 1
                                pb = emit_pair(g, h, pair, psS, "s",
                                               mask_dve=g in (ga, gb))
                                jobs.append(make_pv_job(pair, pb, g, h, pvq,
                                                        state, total))
                                drain(_drain_n)
                        else:
                            for pair, pb in pend:
                                jobs.append(make_pv_job(pair, pb, g, h, pvq,
                                                        state, total))
                                drain(int(os.environ.get("K2_DRAIN_E", "48")))

                        jobs.append(make_hfin_job(g, h, pvq))
                        if h % 2 == 1:
                            jobs.append(make_tr_job(g, h // 2))
                        heads_done[g] += 1
                        if heads_done[g] == HL:
                            for j in range(JB):
                                jobs.append(make_gfin_job(g, j))
                        drain(_drain_n + 1)

                    drain(0)

        for _rep in range(repeat):
            emit_once()

    _split_multi_waits(nc)
    return nc


# ---------------------------------------------------------------------------
# Host entry point
# ---------------------------------------------------------------------------
LAST_EXEC_NS = None
LAST_RESULT = None


def kernel(query, key, value, mask, Wq, Wk, Wv, Wo, bo):
    global LAST_EXEC_NS, LAST_RESULT
    _install_tile_drain_patch()
    from concourse.bass_utils import run_bass_kernel_spmd

    B, S, E = 2, 2048, 1024
    H, D = 16, 64
    N_CORES = 8
    BG = 2                    # batch groups
    HG = N_CORES // BG        # head groups per batch
    HL = H // HG              # heads per core
    DIM = HL * D

    query = np.asarray(query, dtype=np.float32)
    key = np.asarray(key, dtype=np.float32)
    value = np.asarray(value, dtype=np.float32)
    mask2d = np.asarray(mask).reshape(S, S).astype(bool)
    Wq = np.asarray(Wq, dtype=np.float32)
    Wk = np.asarray(Wk, dtype=np.float32)
    Wv = np.asarray(Wv, dtype=np.float32)
    Wo = np.asarray(Wo, dtype=np.float32)
    bo = np.asarray(bo, dtype=np.float32)

    bias_idx, biases, block_live = classify_mask(mask2d, S)
    nuniq = len(biases)
    bias_stack = (np.concatenate(biases, axis=1) if nuniq
                  else np.zeros((128, 128), np.float32))

    nc = build_nc(S, E, D, HL, bias_idx, block_live, nuniq)

    scale = np.float32(1.0 / np.sqrt(D))
    in_maps = []
    for c in range(N_CORES):
        b, hg = c // HG, c % HG
        cols = slice(hg * DIM, (hg + 1) * DIM)
        wv_l = Wv[:, cols].reshape(E, HL, D)
        wv_aug = np.zeros((E, HL, D + 1), np.float32)
        wv_aug[:, :, :D] = wv_l
        in_maps.append({
            "xqT": _bf16(query[b].T),
            "xkT": _bf16(key[b].T),
            "xvT": _bf16(value[b].T),
            "wq": _bf16(Wq[:, cols] * scale),
            "wk": _bf16(Wk[:, cols]),
            "wv": _bf16(wv_aug.reshape(E, HL * (D + 1))),
            "wo": _bf16(Wo[cols, :]),
            "biasT": _bf16(bias_stack),
        })

    res = run_bass_kernel_spmd(nc, in_maps, list(range(N_CORES)))
    LAST_RESULT = res
    LAST_EXEC_NS = res.exec_time_ns or res.mean_exec_time_ns

    out = np.empty((B, S, E), np.float32)
    for b in range(BG):
        acc = res.results[b * HG]["out_p"].astype(np.float32)
        for j in range(1, HG):
            acc = acc + res.results[b * HG + j]["out_p"]
        out[b] = acc + bo[None, :]
    return out


def _bf16(a):
    import ml_dtypes
    return np.ascontiguousarray(np.asarray(a, np.float32)).astype(
        ml_dtypes.bfloat16)
